# revision 2
# baseline (speedup 1.0000x reference)
import sys
import os
import numpy as np

for _p in ("/opt/trn_rl_repo",):
    if _p not in sys.path:
        sys.path.insert(0, _p)

import ml_dtypes

PATCH = 7
STRIDE = 3
GRID = 126  # (384 - 7)//3 + 1
SAMPLE = 64
S = SAMPLE * SAMPLE  # 4096
H_PARAM = 0.5
ORIENT_W = 0.5
OCC_W = 0.05
EPS_NORM = 1e-05

KF = 4                      # Fourier harmonics for |cos| expansion
KC = 3136                   # cosine feature rows
KO = 49 * (2 * KF + 1)      # 441 orientation feature rows
KPAD = 3584                 # 28 * 128
KT = KPAD // 128            # 28
NCORES = 8
MSH = S // NCORES           # 512 rows per core
NCH = 8                     # column chunks of 512
MT = MSH // 128             # 4 m-tiles per core

LAST_EXEC_NS = None
DEVICE_OK = False
_BASS_CACHE = {}


def _grid_idx(field):
    gx = field[..., 0].reshape(-1)
    gy = field[..., 1].reshape(-1)
    ix = np.clip(np.round((gx + 1.0) * GRID / 2.0 - 0.5).astype(np.int64), 0, GRID - 1)
    iy = np.clip(np.round((gy + 1.0) * GRID / 2.0 - 0.5).astype(np.int64), 0, GRID - 1)
    return iy, ix


def _gather_patches(feat, iy, ix):
    # feat [C, H, W] -> [C*49, S] with torch-unfold channel ordering (c*49 + ki*7+kj)
    C = feat.shape[0]
    by = iy * STRIDE
    bx = ix * STRIDE
    out = np.empty((C, PATCH * PATCH, S), dtype=np.float32)
    for ki in range(PATCH):
        for kj in range(PATCH):
            out[:, ki * PATCH + kj, :] = feat[:, by + ki, bx + kj]
    return out.reshape(C * PATCH * PATCH, S)


def _fourier_feats(o):
    # o [98, S]: 2 channels x 49 patch positions. Returns r [49,S], cos/sin harmonics.
    u = o.reshape(2, 49, S)[0]
    v = o.reshape(2, 49, S)[1]
    r2 = u * u + v * v
    r = np.sqrt(r2)
    safe = np.maximum(r2, 1e-30)
    c1 = (u * u - v * v) / safe
    s1 = 2.0 * u * v / safe
    fc, fs = [], []
    ck, sk = c1, s1
    for _k in range(1, KF + 1):
        fc.append(r * ck)
        fs.append(r * sk)
        ck, sk = ck * c1 - sk * s1, sk * c1 + ck * s1
    return r, fc, fs


def _host_features(tf, rf, to, ro):
    """Build FX, FY [KPAD, S] fp32 so that dA = FX.T @ FY."""
    ymean = rf.mean(axis=1, keepdims=True)
    xc = tf - ymean
    yc = rf - ymean
    xn = xc / (np.linalg.norm(xc, axis=0, keepdims=True) + EPS_NORM)
    yn = yc / (np.linalg.norm(yc, axis=0, keepdims=True) + EPS_NORM)

    xs = to.reshape(2, 49, S)
    ys = ro.reshape(2, 49, S)
    R2 = (xs * xs).sum(axis=0).sum(axis=0)  # [S]
    P2 = (ys * ys).sum(axis=0).sum(axis=0)

    rX, fcX, fsX = _fourier_feats(to)
    rY, fcY, fsY = _fourier_feats(ro)
    rows_x = [rX * (2.0 / np.pi)]
    rows_y = [rY]
    for k in range(1, KF + 1):
        coef = (4.0 / np.pi) * ((-1.0) ** (k + 1)) / (4.0 * k * k - 1.0)
        rows_x.append(fcX[k - 1] * coef)
        rows_y.append(fcY[k - 1])
        rows_x.append(fsX[k - 1] * coef)
        rows_y.append(fsY[k - 1])
    AX = np.concatenate(rows_x, axis=0)  # [KO, S]
    AY = np.concatenate(rows_y, axis=0)

    FX = np.zeros((KPAD, S), np.float32)
    FY = np.zeros((KPAD, S), np.float32)
    FX[:KC] = -0.5 * xn
    FY[:KC] = yn
    FX[KC:KC + KO] = -AX / 98.0
    FY[KC:KC + KO] = AY
    FX[KC + KO] = 1.0
    FY[KC + KO] = 0.5 + 0.5 * P2 / 98.0
    FX[KC + KO + 1] = 0.5 * R2 / 98.0
    FY[KC + KO + 1] = 1.0
    return FX, FY


def _cascade_bias(FX, FY):
    """Replace the 2 bf16 bias rows with 4 fp8 residual-cascade rows."""
    import ml_dtypes as _mld

    def q8(a):
        return (a * 16.0).astype(_mld.float8_e4m3).astype(np.float32) / 16.0

    FXz = FX.copy()
    FYz = FY.copy()
    FXz[KC + KO:] = 0.0
    FYz[KC + KO:] = 0.0
    by = FY[KC + KO]       # 0.5 + 0.5*P2/98 (paired with x=1)
    bx = FX[KC + KO + 1]   # 0.5*R2/98      (paired with y=1)
    q1 = q8(by)
    FXz[KC + KO] = 1.0
    FYz[KC + KO] = q1
    FXz[KC + KO + 1] = 1.0
    FYz[KC + KO + 1] = by - q1
    q2 = q8(bx)
    FXz[KC + KO + 2] = q2
    FYz[KC + KO + 2] = 1.0
    FXz[KC + KO + 3] = bx - q2
    FYz[KC + KO + 3] = 1.0
    return FXz, FYz


def _build_bass(stage=6):
    import concourse.bass as bass
    from concourse import mybir, bacc
    from concourse.tile import TileContext

    f32 = mybir.dt.float32
    f32r = mybir.dt.float32r
    bf16 = mybir.dt.bfloat16
    u32 = mybir.dt.uint32
    Alu = mybir.AluOpType
    Act = mybir.ActivationFunctionType

    fp8 = mybir.dt.float8e4
    nc = bacc.Bacc("TRN2", target_bir_lowering=False, debug=False,
                   num_devices=NCORES)
    xw_ext = nc.declare_dram_parameter("xw", [128, KT * 512], fp8, isOutput=False)
    yw_ext = nc.declare_dram_parameter("yw", [NCH, 128, KT * 512], fp8, isOutput=False)
    sout_ext = nc.declare_dram_parameter("sout", [128, 8], f32, isOutput=True)

    with TileContext(nc) as tc:
        with tc.tile_pool(name="xp", bufs=1) as xp, \
             tc.tile_pool(name="yp", bufs=2) as yp, \
             tc.tile_pool(name="dp", bufs=1) as dp, \
             tc.tile_pool(name="sp", bufs=2) as sp, \
             tc.tile_pool(name="hp", bufs=2) as hp, \
             tc.tile_pool(name="pp", bufs=4, space="PSUM") as pp, \
             tc.tile_pool(name="cp", bufs=1, space="PSUM") as cp, \
             tc.tile_pool(name="dr", bufs=2, space="DRAM") as dr:

            xw = xp.tile([128, KT, 512], fp8)
            nc.sync.dma_start(out=xw, in_=xw_ext.rearrange("p (a b) -> p a b", a=KT))

            D = [dp.tile([128, S], bf16, tag=f"D{m}", name=f"D{m}") for m in range(MT)]
            mnc = sp.tile([128, MT * NCH], f32, tag="mnc", bufs=1)   # chunk mins
            sout = sp.tile([128, 8], f32, tag="sout", bufs=1)
            m0 = sp.tile([128, MT], f32, tag="m0", bufs=1)
            m0b = sp.tile([128, MT], bf16, tag="m0b", bufs=1)
            amu = sp.tile([128, MT], mybir.dt.uint16, tag="amu", bufs=1)

            # ---- Phase 1: fused GEMM dA = FX.T @ FY (fp8 DoubleRow + bf16 bias) ----
            KH = KT // 2
            for n in range(NCH):
                y = yp.tile([128, KT, 512], fp8, tag="y")
                yv = yw_ext[n].rearrange("p (a b) -> p a b", a=KT)
                nc.sync.dma_start(out=y[:, 0:KH, :], in_=yv[:, 0:KH, :])
                nc.gpsimd.dma_start(out=y[:, KH:KT, :], in_=yv[:, KH:KT, :])
                for m in range(MT):
                    ps = pp.tile([128, 512], f32, tag="ps")
                    for k in range(KT // 2):
                        nc.tensor.matmul(
                            out=ps,
                            lhsT=xw[:, 2 * k:2 * k + 2, m * 128:(m + 1) * 128],
                            rhs=y[:, 2 * k:2 * k + 2, :],
                            start=(k == 0),
                            stop=(k == KT // 2 - 1),
                            perf_mode=mybir.MatmulPerfMode.DoubleRow,
                        )
                    nc.scalar.activation(
                        out=D[m][:, n * 512:(n + 1) * 512], in_=ps,
                        func=Act.Copy, bias=0.0, scale=1.0 / 256.0)
                    mn = m * NCH + n
                    nc.vector.tensor_reduce(
                        out=mnc[:, mn:mn + 1],
                        in_=D[m][:, n * 512:(n + 1) * 512],
                        axis=mybir.AxisListType.X, op=Alu.min)
                    if stage >= 2 and n == NCH - 1:
                        nc.vector.tensor_reduce(
                            out=m0[:, m:m + 1], in_=mnc[:, m * NCH:(m + 1) * NCH],
                            axis=mybir.AxisListType.X, op=Alu.min)
                        nc.vector.tensor_copy(out=m0b[:, m:m + 1],
                                              in_=m0[:, m:m + 1])
                        amidx8 = hp.tile([128, 8], mybir.dt.uint16, tag="amidx8")
                        nc.vector.max_index(
                            out=amidx8,
                            in_max=m0b[:, m:m + 1].broadcast_to([128, 8]),
                            in_values=D[m])
                        nc.vector.tensor_copy(out=amu[:, m:m + 1], in_=amidx8[:, 0:1])

            if stage < 2:
                for m in range(MT):
                    nc.vector.tensor_reduce(
                        out=sout[:, m:m + 1], in_=mnc[:, m * NCH:(m + 1) * NCH],
                        axis=mybir.AxisListType.X, op=Alu.min)
                    nc.vector.tensor_copy(out=sout[:, 4 + m:5 + m],
                                          in_=mnc[:, m * NCH:m * NCH + 1])

            if stage >= 2:
                # ---- Phase 2: allgather argmins, histogram ----
                am_loc = dr.tile([MSH], mybir.dt.uint16)
                nc.sync.dma_start(out=am_loc.rearrange("(m p) -> p m", m=MT),
                                  in_=amu)
                am_all = dr.tile([S], mybir.dt.uint16)
                nc.gpsimd.collective_compute(
                    "AllGather", Alu.bypass,
                    replica_groups=[list(range(NCORES))],
                    ins=[am_loc[:].opt()], outs=[am_all[:].opt()],
                )
                amsb = hp.tile([128, 32], mybir.dt.uint16, tag="amsb")
                nc.sync.dma_start(out=amsb, in_=am_all.rearrange("(p f) -> p f", p=128))

                hi = hp.tile([128, 32], mybir.dt.uint16, tag="hi")
                lo = hp.tile([128, 32], mybir.dt.uint16, tag="lo")
                nc.vector.tensor_scalar(out=hi, in0=amsb, scalar1=5, scalar2=None,
                                        op0=Alu.logical_shift_right)
                nc.vector.tensor_scalar(out=lo, in0=amsb, scalar1=31, scalar2=None,
                                        op0=Alu.bitwise_and)
                hif = hp.tile([128, 32], f32, tag="hif")
                lof = hp.tile([128, 32], f32, tag="lof")
                nc.vector.tensor_copy(out=hif, in_=hi)
                nc.vector.tensor_copy(out=lof, in_=lo)

                iota_a = hp.tile([128, 128], f32, tag="iota_a", bufs=1)
                nc.gpsimd.iota(iota_a, pattern=[[1, 128]], base=0,
                               channel_multiplier=0,
                               allow_small_or_imprecise_dtypes=True)
                iota_b = hp.tile([128, 32], f32, tag="iota_b", bufs=1)
                nc.gpsimd.iota(iota_b, pattern=[[1, 32]], base=0,
                               channel_multiplier=0,
                               allow_small_or_imprecise_dtypes=True)

                Hall = hp.tile([128, 32, 128], bf16, tag="Hall", bufs=1)
                Lall = hp.tile([128, 32, 32], bf16, tag="Lall", bufs=1)
                nc.vector.tensor_tensor(
                    out=Hall,
                    in0=hif.rearrange("p (f o) -> p f o", o=1)
                        .broadcast_to([128, 32, 128]),
                    in1=iota_a.rearrange("p (o a) -> p o a", o=1)
                        .broadcast_to([128, 32, 128]),
                    op=Alu.is_equal)
                nc.vector.tensor_tensor(
                    out=Lall,
                    in0=lof.rearrange("p (f o) -> p f o", o=1)
                        .broadcast_to([128, 32, 32]),
                    in1=iota_b.rearrange("p (o b) -> p o b", o=1)
                        .broadcast_to([128, 32, 32]),
                    op=Alu.is_equal)

                cnt_ps = cp.tile([128, 32], f32, tag="cnt")
                for f in range(32):
                    nc.tensor.matmul(out=cnt_ps, lhsT=Hall[:, f, :],
                                     rhs=Lall[:, f, :],
                                     start=(f == 0), stop=(f == 31))

            if stage == 2:
                csb2 = hp.tile([128, 32], f32, tag="csb2", bufs=1)
                nc.scalar.copy(out=csb2, in_=cnt_ps)
                nc.vector.tensor_copy(out=sout[:, 0:8], in_=csb2[:, 0:8])

            if stage >= 3:
                # counts [128a, 32b] (t = 32a + b) -> pen_row [1, 4096] f32
                csb = hp.tile([128, 32], f32, tag="csb", bufs=1)
                nc.scalar.copy(out=csb, in_=cnt_ps)
                cnt_dram = dr.tile([S], f32)
                nc.sync.dma_start(out=cnt_dram.rearrange("(p f) -> p f", p=128),
                                  in_=csb)
                pen_row = hp.tile([1, S], f32, tag="pen_row", bufs=1)
                nc.sync.dma_start(out=pen_row,
                                  in_=cnt_dram.rearrange("(o s) -> o s", o=1))
                w005 = hp.tile([1, 128], f32, tag="w005", bufs=1)
                nc.vector.memset(w005, OCC_W)
                # pen broadcast tiles [128, 4096] bf16
                pen_bc = sp.tile([128, S], bf16, tag="pen_bc", bufs=1)
                for n in range(NCH):
                    pen_ps = pp.tile([128, 512], f32, tag="ps")
                    nc.tensor.matmul(out=pen_ps,
                                     lhsT=w005.bitcast(f32r),
                                     rhs=pen_row[:, n * 512:(n + 1) * 512]
                                         .bitcast(f32r),
                                     start=True, stop=True,
                                     skip_group_check=True)
                    nc.scalar.copy(out=pen_bc[:, n * 512:(n + 1) * 512], in_=pen_ps)

            if stage >= 5:
                # ---- Phase 3: Dp = dA + pen, row min + exp row sums ----
                mfin = sp.tile([128, MT], f32, tag="mfin", bufs=1)
                it4 = sp.tile([128, MT], f32, tag="it4", bufs=1)
                nit4 = sp.tile([128, MT], f32, tag="nit4", bufs=1)
                bia4 = sp.tile([128, MT], f32, tag="bia4", bufs=1)
                half = sp.tile([128, MT], f32, tag="half", bufs=1)
                for m in range(MT):
                    nc.vector.tensor_tensor(out=D[m], in0=D[m], in1=pen_bc,
                                            op=Alu.add)
                    nc.vector.tensor_reduce(
                        out=mfin[:, m:m + 1], in_=D[m],
                        axis=mybir.AxisListType.X, op=Alu.min)
                    nc.vector.tensor_scalar(out=half[:, m:m + 1],
                                            in0=mfin[:, m:m + 1], scalar1=1e-5,
                                            scalar2=H_PARAM,
                                            op0=Alu.add, op1=Alu.mult)
                    nc.vector.reciprocal(out=it4[:, m:m + 1], in_=half[:, m:m + 1])
                    nc.vector.tensor_scalar(out=nit4[:, m:m + 1],
                                            in0=it4[:, m:m + 1], scalar1=-1.0,
                                            scalar2=None, op0=Alu.mult)
                    nc.vector.tensor_tensor(out=bia4[:, m:m + 1],
                                            in0=mfin[:, m:m + 1],
                                            in1=it4[:, m:m + 1], op=Alu.mult)
                    ed = sp.tile([128, S], bf16, tag="ed")
                    nc.scalar.activation(
                        out=ed, in_=D[m],
                        func=Act.Exp,
                        bias=bia4[:, m:m + 1], scale=nit4[:, m:m + 1],
                        accum_out=sout[:, m:m + 1])
                nc.vector.tensor_copy(out=sout[:, 4:4 + MT], in_=mfin)

            nc.sync.dma_start(out=sout_ext[:], in_=sout)

    nc.compile()
    return nc


def _pack_inputs(target_features, reference_features, target_orient, refer_orient,
                 target_field, refer_field):
    iy_t, ix_t = _grid_idx(np.asarray(target_field[0], dtype=np.float32))
    iy_r, ix_r = _grid_idx(np.asarray(refer_field[0], dtype=np.float32))

    tf = _gather_patches(np.asarray(target_features[0], np.float32), iy_t, ix_t)
    rf = _gather_patches(np.asarray(reference_features[0], np.float32), iy_r, ix_r)
    to = _gather_patches(np.asarray(target_orient[0], np.float32), iy_t, ix_t)
    ro = _gather_patches(np.asarray(refer_orient[0], np.float32), iy_r, ix_r)

    FX, FY = _host_features(tf, rf, to, ro)
    FXz, FYz = _cascade_bias(FX, FY)
    FXq = (FXz * 16.0).astype(ml_dtypes.float8_e4m3).reshape(KT, 128, S)
    FYq = (FYz * 16.0).astype(ml_dtypes.float8_e4m3).reshape(KT, 128, NCH, 512)
    yw = np.ascontiguousarray(FYq.transpose(2, 1, 0, 3)).reshape(NCH, 128, KT * 512)
    in_maps = []
    for c in range(NCORES):
        xw_c = np.ascontiguousarray(
            FXq[:, :, c * MSH:(c + 1) * MSH].transpose(1, 0, 2)
        ).reshape(128, KT * 512)
        in_maps.append({"xw": xw_c, "yw": yw})
    return in_maps, (FXz, FYz)


def kernel(target_features, reference_features, target_orient, refer_orient,
           target_field, refer_field):
    global DEVICE_OK, LAST_EXEC_NS
    in_maps, (FXz, FYz) = _pack_inputs(target_features, reference_features,
                                       target_orient, refer_orient,
                                       target_field, refer_field)
    try:
        from concourse.bass_utils import run_bass_kernel_spmd
        if "nc" not in _BASS_CACHE:
            _BASS_CACHE["nc"] = _build_bass()
        nc = _BASS_CACHE["nc"]
        res = run_bass_kernel_spmd(nc, in_maps, list(range(NCORES)))
        LAST_EXEC_NS = getattr(res, "exec_time_ns", None)
        S_all = np.empty(S, np.float32)
        for c in range(NCORES):
            o = res.results[c]["sout"]  # [128, 8]
            for m in range(MT):
                S_all[c * MSH + m * 128:c * MSH + (m + 1) * 128] = o[:, m]
        DEVICE_OK = True
        loss = np.log(S_all).mean()
        return np.float32(loss)
    except Exception:
        sys.stderr.write("device path failed; host fallback\n")
        import traceback
        traceback.print_exc()
        import ml_dtypes as _mld
        FXe = (FXz * 16.0).astype(_mld.float8_e4m3).astype(np.float32) / 16.0
        FYe = (FYz * 16.0).astype(_mld.float8_e4m3).astype(np.float32) / 16.0
        dA = FXe.T @ FYe
        am = np.argmin(dA, axis=1)
        counts = np.bincount(am, minlength=S).astype(np.float32)
        dtot = dA + OCC_W * counts[None, :]
        m = dtot.min(axis=1)
        it = 1.0 / (H_PARAM * (m + 1e-5))
        Srow = np.exp((m[:, None] - dtot) * it[:, None]).sum(axis=1)
        return np.float32(np.log(Srow).mean())


# revision 3
# speedup vs baseline: 1.0459x; 1.0459x over previous
import sys
import os
import numpy as np

for _p in ("/opt/trn_rl_repo",):
    if _p not in sys.path:
        sys.path.insert(0, _p)

import ml_dtypes

PATCH = 7
STRIDE = 3
GRID = 126  # (384 - 7)//3 + 1
SAMPLE = 64
S = SAMPLE * SAMPLE  # 4096
H_PARAM = 0.5
ORIENT_W = 0.5
OCC_W = 0.05
EPS_NORM = 1e-05

KF = 4                      # Fourier harmonics for |cos| expansion
KC = 3136                   # cosine feature rows
KO = 49 * (2 * KF + 1)      # 441 orientation feature rows
KPAD = 3584                 # 28 * 128
KT = KPAD // 128            # 28
NCORES = 8
MSH = S // NCORES           # 512 rows per core
NCH = 8                     # column chunks of 512
MT = MSH // 128             # 4 m-tiles per core

LAST_EXEC_NS = None
DEVICE_OK = False
_BASS_CACHE = {}


def _grid_idx(field):
    gx = field[..., 0].reshape(-1)
    gy = field[..., 1].reshape(-1)
    ix = np.clip(np.round((gx + 1.0) * GRID / 2.0 - 0.5).astype(np.int64), 0, GRID - 1)
    iy = np.clip(np.round((gy + 1.0) * GRID / 2.0 - 0.5).astype(np.int64), 0, GRID - 1)
    return iy, ix


def _gather_patches(feat, iy, ix):
    # feat [C, H, W] -> [C*49, S] with torch-unfold channel ordering (c*49 + ki*7+kj)
    C = feat.shape[0]
    by = iy * STRIDE
    bx = ix * STRIDE
    out = np.empty((C, PATCH * PATCH, S), dtype=np.float32)
    for ki in range(PATCH):
        for kj in range(PATCH):
            out[:, ki * PATCH + kj, :] = feat[:, by + ki, bx + kj]
    return out.reshape(C * PATCH * PATCH, S)


def _fourier_feats(o):
    # o [98, S]: 2 channels x 49 patch positions. Returns r [49,S], cos/sin harmonics.
    u = o.reshape(2, 49, S)[0]
    v = o.reshape(2, 49, S)[1]
    r2 = u * u + v * v
    r = np.sqrt(r2)
    safe = np.maximum(r2, 1e-30)
    c1 = (u * u - v * v) / safe
    s1 = 2.0 * u * v / safe
    fc, fs = [], []
    ck, sk = c1, s1
    for _k in range(1, KF + 1):
        fc.append(r * ck)
        fs.append(r * sk)
        ck, sk = ck * c1 - sk * s1, sk * c1 + ck * s1
    return r, fc, fs


def _host_features(tf, rf, to, ro):
    """Build FX, FY [KPAD, S] fp32 so that dA = FX.T @ FY."""
    ymean = rf.mean(axis=1, keepdims=True)
    xc = tf - ymean
    yc = rf - ymean
    xn = xc / (np.linalg.norm(xc, axis=0, keepdims=True) + EPS_NORM)
    yn = yc / (np.linalg.norm(yc, axis=0, keepdims=True) + EPS_NORM)

    xs = to.reshape(2, 49, S)
    ys = ro.reshape(2, 49, S)
    R2 = (xs * xs).sum(axis=0).sum(axis=0)  # [S]
    P2 = (ys * ys).sum(axis=0).sum(axis=0)

    rX, fcX, fsX = _fourier_feats(to)
    rY, fcY, fsY = _fourier_feats(ro)
    rows_x = [rX * (2.0 / np.pi)]
    rows_y = [rY]
    for k in range(1, KF + 1):
        coef = (4.0 / np.pi) * ((-1.0) ** (k + 1)) / (4.0 * k * k - 1.0)
        rows_x.append(fcX[k - 1] * coef)
        rows_y.append(fcY[k - 1])
        rows_x.append(fsX[k - 1] * coef)
        rows_y.append(fsY[k - 1])
    AX = np.concatenate(rows_x, axis=0)  # [KO, S]
    AY = np.concatenate(rows_y, axis=0)

    FX = np.zeros((KPAD, S), np.float32)
    FY = np.zeros((KPAD, S), np.float32)
    FX[:KC] = -0.5 * xn
    FY[:KC] = yn
    FX[KC:KC + KO] = -AX / 98.0
    FY[KC:KC + KO] = AY
    FX[KC + KO] = 1.0
    FY[KC + KO] = 0.5 + 0.5 * P2 / 98.0
    FX[KC + KO + 1] = 0.5 * R2 / 98.0
    FY[KC + KO + 1] = 1.0
    return FX, FY


def _cascade_bias(FX, FY):
    """Replace the 2 bf16 bias rows with 4 fp8 residual-cascade rows."""
    import ml_dtypes as _mld

    def q8(a):
        return (a * 16.0).astype(_mld.float8_e4m3).astype(np.float32) / 16.0

    FXz = FX.copy()
    FYz = FY.copy()
    FXz[KC + KO:] = 0.0
    FYz[KC + KO:] = 0.0
    by = FY[KC + KO]       # 0.5 + 0.5*P2/98 (paired with x=1)
    bx = FX[KC + KO + 1]   # 0.5*R2/98      (paired with y=1)
    q1 = q8(by)
    FXz[KC + KO] = 1.0
    FYz[KC + KO] = q1
    FXz[KC + KO + 1] = 1.0
    FYz[KC + KO + 1] = by - q1
    q2 = q8(bx)
    FXz[KC + KO + 2] = q2
    FYz[KC + KO + 2] = 1.0
    FXz[KC + KO + 3] = bx - q2
    FYz[KC + KO + 3] = 1.0
    return FXz, FYz


def _build_bass(stage=6):
    import concourse.bass as bass
    from concourse import mybir, bacc
    from concourse.tile import TileContext

    f32 = mybir.dt.float32
    f32r = mybir.dt.float32r
    bf16 = mybir.dt.bfloat16
    u32 = mybir.dt.uint32
    Alu = mybir.AluOpType
    Act = mybir.ActivationFunctionType

    fp8 = mybir.dt.float8e4
    nc = bacc.Bacc("TRN2", target_bir_lowering=False, debug=False,
                   num_devices=NCORES)
    xw_ext = nc.declare_dram_parameter("xw", [128, KT * 512], fp8, isOutput=False)
    yw_ext = nc.declare_dram_parameter("yw", [NCH, 128, KT * 512], fp8, isOutput=False)
    sout_ext = nc.declare_dram_parameter("sout", [128, 8], f32, isOutput=True)

    with TileContext(nc) as tc:
        with tc.tile_pool(name="xp", bufs=1) as xp, \
             tc.tile_pool(name="yp", bufs=3) as yp, \
             tc.tile_pool(name="dp", bufs=1) as dp, \
             tc.tile_pool(name="sp", bufs=2) as sp, \
             tc.tile_pool(name="hp", bufs=2) as hp, \
             tc.tile_pool(name="pp", bufs=6, space="PSUM") as pp, \
             tc.tile_pool(name="cp", bufs=1, space="PSUM") as cp, \
             tc.tile_pool(name="dr", bufs=2, space="DRAM") as dr:

            xw = xp.tile([128, KT, 512], fp8)
            xv = xw_ext.rearrange("p (a b) -> p a b", a=KT)
            nc.sync.dma_start(out=xw[:, 0:KT // 2, :], in_=xv[:, 0:KT // 2, :])
            nc.gpsimd.dma_start(out=xw[:, KT // 2:KT, :], in_=xv[:, KT // 2:KT, :])

            D = [dp.tile([128, S], bf16, tag=f"D{m}", name=f"D{m}") for m in range(MT)]
            mnc = sp.tile([128, MT * NCH], f32, tag="mnc", bufs=1)   # chunk mins
            sout = sp.tile([128, 8], f32, tag="sout", bufs=1)
            m0 = sp.tile([128, MT], f32, tag="m0", bufs=1)
            m0b = sp.tile([128, MT], bf16, tag="m0b", bufs=1)
            amu = sp.tile([128, MT], mybir.dt.uint16, tag="amu", bufs=1)

            # ---- Phase 1: fused GEMM dA = FX.T @ FY (fp8 DoubleRow + bf16 bias) ----
            KQ = KT // 4
            for n in range(NCH):
                y = yp.tile([128, KT, 512], fp8, tag="y")
                yv = yw_ext[n].rearrange("p (a b) -> p a b", a=KT)
                for qi, eng in ((0, nc.sync), (1, nc.gpsimd), (2, nc.sync), (3, nc.gpsimd)):
                    eng.dma_start(out=y[:, qi * KQ:(qi + 1) * KQ, :],
                                  in_=yv[:, qi * KQ:(qi + 1) * KQ, :])
                for m in range(MT):
                    ps = pp.tile([128, 512], f32, tag="ps")
                    for k in range(KT // 2):
                        nc.tensor.matmul(
                            out=ps,
                            lhsT=xw[:, 2 * k:2 * k + 2, m * 128:(m + 1) * 128],
                            rhs=y[:, 2 * k:2 * k + 2, :],
                            start=(k == 0),
                            stop=(k == KT // 2 - 1),
                            perf_mode=mybir.MatmulPerfMode.DoubleRow,
                        )
                    nc.scalar.activation(
                        out=D[m][:, n * 512:(n + 1) * 512], in_=ps,
                        func=Act.Copy, bias=0.0, scale=1.0 / 256.0)
                    mn = m * NCH + n
                    nc.vector.tensor_reduce(
                        out=mnc[:, mn:mn + 1],
                        in_=D[m][:, n * 512:(n + 1) * 512],
                        axis=mybir.AxisListType.X, op=Alu.min)
                    if stage >= 2 and n == NCH - 1:
                        nc.vector.tensor_reduce(
                            out=m0[:, m:m + 1], in_=mnc[:, m * NCH:(m + 1) * NCH],
                            axis=mybir.AxisListType.X, op=Alu.min)
                        mn8b = hp.tile([128, 8], bf16, tag="mn8b")
                        nc.vector.tensor_scalar(
                            out=mn8b, in0=mnc[:, m * NCH:(m + 1) * NCH],
                            scalar1=m0[:, m:m + 1], scalar2=None, op0=Alu.min)
                        amidx8 = hp.tile([128, 8], mybir.dt.uint16, tag="amidx8")
                        nc.vector.max_index(
                            out=amidx8,
                            in_max=mn8b,
                            in_values=D[m])
                        nc.vector.tensor_copy(out=amu[:, m:m + 1], in_=amidx8[:, 0:1])

            if stage < 2:
                for m in range(MT):
                    nc.vector.tensor_reduce(
                        out=sout[:, m:m + 1], in_=mnc[:, m * NCH:(m + 1) * NCH],
                        axis=mybir.AxisListType.X, op=Alu.min)
                    nc.vector.tensor_copy(out=sout[:, 4 + m:5 + m],
                                          in_=mnc[:, m * NCH:m * NCH + 1])

            if stage >= 2:
                # ---- Phase 2: allgather argmins, histogram ----
                am_loc = dr.tile([MSH], mybir.dt.uint16)
                nc.sync.dma_start(out=am_loc.rearrange("(m p) -> p m", m=MT),
                                  in_=amu)
                am_all = dr.tile([S], mybir.dt.uint16)
                nc.gpsimd.collective_compute(
                    "AllGather", Alu.bypass,
                    replica_groups=[list(range(NCORES))],
                    ins=[am_loc[:].opt()], outs=[am_all[:].opt()],
                )
                amsb = hp.tile([128, 32], mybir.dt.uint16, tag="amsb")
                nc.sync.dma_start(out=amsb, in_=am_all.rearrange("(p f) -> p f", p=128))

                hi = hp.tile([128, 32], mybir.dt.uint16, tag="hi")
                lo = hp.tile([128, 32], mybir.dt.uint16, tag="lo")
                nc.vector.tensor_scalar(out=hi, in0=amsb, scalar1=5, scalar2=None,
                                        op0=Alu.logical_shift_right)
                nc.vector.tensor_scalar(out=lo, in0=amsb, scalar1=31, scalar2=None,
                                        op0=Alu.bitwise_and)
                hif = hp.tile([128, 32], f32, tag="hif")
                lof = hp.tile([128, 32], f32, tag="lof")
                nc.vector.tensor_copy(out=hif, in_=hi)
                nc.vector.tensor_copy(out=lof, in_=lo)

                iota_a = hp.tile([128, 128], f32, tag="iota_a", bufs=1)
                nc.gpsimd.iota(iota_a, pattern=[[1, 128]], base=0,
                               channel_multiplier=0,
                               allow_small_or_imprecise_dtypes=True)
                iota_b = hp.tile([128, 32], f32, tag="iota_b", bufs=1)
                nc.gpsimd.iota(iota_b, pattern=[[1, 32]], base=0,
                               channel_multiplier=0,
                               allow_small_or_imprecise_dtypes=True)

                Hall = hp.tile([128, 32, 128], bf16, tag="Hall", bufs=1)
                Lall = hp.tile([128, 32, 32], bf16, tag="Lall", bufs=1)
                nc.vector.tensor_tensor(
                    out=Hall,
                    in0=hif.rearrange("p (f o) -> p f o", o=1)
                        .broadcast_to([128, 32, 128]),
                    in1=iota_a.rearrange("p (o a) -> p o a", o=1)
                        .broadcast_to([128, 32, 128]),
                    op=Alu.is_equal)
                nc.vector.tensor_tensor(
                    out=Lall,
                    in0=lof.rearrange("p (f o) -> p f o", o=1)
                        .broadcast_to([128, 32, 32]),
                    in1=iota_b.rearrange("p (o b) -> p o b", o=1)
                        .broadcast_to([128, 32, 32]),
                    op=Alu.is_equal)

                cnt_ps = cp.tile([128, 32], f32, tag="cnt")
                for f in range(32):
                    nc.tensor.matmul(out=cnt_ps, lhsT=Hall[:, f, :],
                                     rhs=Lall[:, f, :],
                                     start=(f == 0), stop=(f == 31))

            if stage == 2:
                csb2 = hp.tile([128, 32], f32, tag="csb2", bufs=1)
                nc.scalar.copy(out=csb2, in_=cnt_ps)
                nc.vector.tensor_copy(out=sout[:, 0:8], in_=csb2[:, 0:8])

            if stage >= 3:
                # counts [128a, 32b] (t = 32a + b) -> pen_row [1, 4096] f32
                csb = hp.tile([128, 32], f32, tag="csb", bufs=1)
                nc.scalar.copy(out=csb, in_=cnt_ps)
                cnt_dram = dr.tile([S], f32)
                nc.sync.dma_start(out=cnt_dram.rearrange("(p f) -> p f", p=128),
                                  in_=csb)
                pen_row = hp.tile([1, S], f32, tag="pen_row", bufs=1)
                nc.sync.dma_start(out=pen_row,
                                  in_=cnt_dram.rearrange("(o s) -> o s", o=1))
                w005 = hp.tile([1, 128], f32, tag="w005", bufs=1)
                nc.vector.memset(w005, OCC_W)
                # pen broadcast tiles [128, 4096] bf16
                pen_bc = sp.tile([128, S], bf16, tag="pen_bc", bufs=1)
                for n in range(NCH):
                    pen_ps = pp.tile([128, 512], f32, tag="ps")
                    nc.tensor.matmul(out=pen_ps,
                                     lhsT=w005.bitcast(f32r),
                                     rhs=pen_row[:, n * 512:(n + 1) * 512]
                                         .bitcast(f32r),
                                     start=True, stop=True,
                                     skip_group_check=True)
                    nc.scalar.copy(out=pen_bc[:, n * 512:(n + 1) * 512], in_=pen_ps)

            if stage >= 5:
                # ---- Phase 3: Dp = dA + pen, row min + exp row sums ----
                mfin = sp.tile([128, MT], f32, tag="mfin", bufs=1)
                it4 = sp.tile([128, MT], f32, tag="it4", bufs=1)
                nit4 = sp.tile([128, MT], f32, tag="nit4", bufs=1)
                bia4 = sp.tile([128, MT], f32, tag="bia4", bufs=1)
                half = sp.tile([128, MT], f32, tag="half", bufs=1)
                for m in range(MT):
                    nc.vector.tensor_tensor(out=D[m], in0=D[m], in1=pen_bc,
                                            op=Alu.add)
                    nc.vector.tensor_reduce(
                        out=mfin[:, m:m + 1], in_=D[m],
                        axis=mybir.AxisListType.X, op=Alu.min)
                    nc.vector.tensor_scalar(out=half[:, m:m + 1],
                                            in0=mfin[:, m:m + 1], scalar1=1e-5,
                                            scalar2=H_PARAM,
                                            op0=Alu.add, op1=Alu.mult)
                    nc.vector.reciprocal(out=it4[:, m:m + 1], in_=half[:, m:m + 1])
                    nc.vector.tensor_scalar(out=nit4[:, m:m + 1],
                                            in0=it4[:, m:m + 1], scalar1=-1.0,
                                            scalar2=None, op0=Alu.mult)
                    nc.vector.tensor_tensor(out=bia4[:, m:m + 1],
                                            in0=mfin[:, m:m + 1],
                                            in1=it4[:, m:m + 1], op=Alu.mult)
                    ed = sp.tile([128, S], bf16, tag="ed")
                    nc.scalar.activation(
                        out=ed, in_=D[m],
                        func=Act.Exp,
                        bias=bia4[:, m:m + 1], scale=nit4[:, m:m + 1],
                        accum_out=sout[:, m:m + 1])
                nc.vector.tensor_copy(out=sout[:, 4:4 + MT], in_=mfin)

            nc.sync.dma_start(out=sout_ext[:], in_=sout)

    nc.compile()
    return nc


def _pack_inputs(target_features, reference_features, target_orient, refer_orient,
                 target_field, refer_field):
    iy_t, ix_t = _grid_idx(np.asarray(target_field[0], dtype=np.float32))
    iy_r, ix_r = _grid_idx(np.asarray(refer_field[0], dtype=np.float32))

    tf = _gather_patches(np.asarray(target_features[0], np.float32), iy_t, ix_t)
    rf = _gather_patches(np.asarray(reference_features[0], np.float32), iy_r, ix_r)
    to = _gather_patches(np.asarray(target_orient[0], np.float32), iy_t, ix_t)
    ro = _gather_patches(np.asarray(refer_orient[0], np.float32), iy_r, ix_r)

    FX, FY = _host_features(tf, rf, to, ro)
    FXz, FYz = _cascade_bias(FX, FY)
    FXq = (FXz * 16.0).astype(ml_dtypes.float8_e4m3).reshape(KT, 128, S)
    FYq = (FYz * 16.0).astype(ml_dtypes.float8_e4m3).reshape(KT, 128, NCH, 512)
    yw = np.ascontiguousarray(FYq.transpose(2, 1, 0, 3)).reshape(NCH, 128, KT * 512)
    in_maps = []
    for c in range(NCORES):
        xw_c = np.ascontiguousarray(
            FXq[:, :, c * MSH:(c + 1) * MSH].transpose(1, 0, 2)
        ).reshape(128, KT * 512)
        in_maps.append({"xw": xw_c, "yw": yw})
    return in_maps, (FXz, FYz)


def kernel(target_features, reference_features, target_orient, refer_orient,
           target_field, refer_field):
    global DEVICE_OK, LAST_EXEC_NS
    in_maps, (FXz, FYz) = _pack_inputs(target_features, reference_features,
                                       target_orient, refer_orient,
                                       target_field, refer_field)
    try:
        from concourse.bass_utils import run_bass_kernel_spmd
        if "nc" not in _BASS_CACHE:
            _BASS_CACHE["nc"] = _build_bass()
        nc = _BASS_CACHE["nc"]
        res = run_bass_kernel_spmd(nc, in_maps, list(range(NCORES)))
        LAST_EXEC_NS = getattr(res, "exec_time_ns", None)
        S_all = np.empty(S, np.float32)
        for c in range(NCORES):
            o = res.results[c]["sout"]  # [128, 8]
            for m in range(MT):
                S_all[c * MSH + m * 128:c * MSH + (m + 1) * 128] = o[:, m]
        DEVICE_OK = True
        loss = np.log(S_all).mean()
        return np.float32(loss)
    except Exception:
        sys.stderr.write("device path failed; host fallback\n")
        import traceback
        traceback.print_exc()
        import ml_dtypes as _mld
        FXe = (FXz * 16.0).astype(_mld.float8_e4m3).astype(np.float32) / 16.0
        FYe = (FYz * 16.0).astype(_mld.float8_e4m3).astype(np.float32) / 16.0
        dA = FXe.T @ FYe
        am = np.argmin(dA, axis=1)
        counts = np.bincount(am, minlength=S).astype(np.float32)
        dtot = dA + OCC_W * counts[None, :]
        m = dtot.min(axis=1)
        it = 1.0 / (H_PARAM * (m + 1e-5))
        Srow = np.exp((m[:, None] - dtot) * it[:, None]).sum(axis=1)
        return np.float32(np.log(Srow).mean())


# revision 4
# speedup vs baseline: 1.1039x; 1.0554x over previous
import sys
import os
import numpy as np

for _p in ("/opt/trn_rl_repo",):
    if _p not in sys.path:
        sys.path.insert(0, _p)

import ml_dtypes

PATCH = 7
STRIDE = 3
GRID = 126  # (384 - 7)//3 + 1
SAMPLE = 64
S = SAMPLE * SAMPLE  # 4096
H_PARAM = 0.5
ORIENT_W = 0.5
OCC_W = 0.05
EPS_NORM = 1e-05

KF = 4                      # Fourier harmonics for |cos| expansion
KC = 3136                   # cosine feature rows
KO = 49 * (2 * KF + 1)      # 441 orientation feature rows
KPAD = 3584                 # 28 * 128
KT = KPAD // 128            # 28
NCORES = 8
MSH = S // NCORES           # 512 rows per core
NCH = 8                     # column chunks of 512
MT = MSH // 128             # 4 m-tiles per core

LAST_EXEC_NS = None
DEVICE_OK = False
_BASS_CACHE = {}


def _grid_idx(field):
    gx = field[..., 0].reshape(-1)
    gy = field[..., 1].reshape(-1)
    ix = np.clip(np.round((gx + 1.0) * GRID / 2.0 - 0.5).astype(np.int64), 0, GRID - 1)
    iy = np.clip(np.round((gy + 1.0) * GRID / 2.0 - 0.5).astype(np.int64), 0, GRID - 1)
    return iy, ix


def _gather_patches(feat, iy, ix):
    # feat [C, H, W] -> [C*49, S] with torch-unfold channel ordering (c*49 + ki*7+kj)
    C = feat.shape[0]
    by = iy * STRIDE
    bx = ix * STRIDE
    out = np.empty((C, PATCH * PATCH, S), dtype=np.float32)
    for ki in range(PATCH):
        for kj in range(PATCH):
            out[:, ki * PATCH + kj, :] = feat[:, by + ki, bx + kj]
    return out.reshape(C * PATCH * PATCH, S)


def _fourier_feats(o):
    # o [98, S]: 2 channels x 49 patch positions. Returns r [49,S], cos/sin harmonics.
    u = o.reshape(2, 49, S)[0]
    v = o.reshape(2, 49, S)[1]
    r2 = u * u + v * v
    r = np.sqrt(r2)
    safe = np.maximum(r2, 1e-30)
    c1 = (u * u - v * v) / safe
    s1 = 2.0 * u * v / safe
    fc, fs = [], []
    ck, sk = c1, s1
    for _k in range(1, KF + 1):
        fc.append(r * ck)
        fs.append(r * sk)
        ck, sk = ck * c1 - sk * s1, sk * c1 + ck * s1
    return r, fc, fs


def _host_features(tf, rf, to, ro):
    """Build FX, FY [KPAD, S] fp32 so that dA = FX.T @ FY."""
    ymean = rf.mean(axis=1, keepdims=True)
    xc = tf - ymean
    yc = rf - ymean
    xn = xc / (np.linalg.norm(xc, axis=0, keepdims=True) + EPS_NORM)
    yn = yc / (np.linalg.norm(yc, axis=0, keepdims=True) + EPS_NORM)

    xs = to.reshape(2, 49, S)
    ys = ro.reshape(2, 49, S)
    R2 = (xs * xs).sum(axis=0).sum(axis=0)  # [S]
    P2 = (ys * ys).sum(axis=0).sum(axis=0)

    rX, fcX, fsX = _fourier_feats(to)
    rY, fcY, fsY = _fourier_feats(ro)
    rows_x = [rX * (2.0 / np.pi)]
    rows_y = [rY]
    for k in range(1, KF + 1):
        coef = (4.0 / np.pi) * ((-1.0) ** (k + 1)) / (4.0 * k * k - 1.0)
        rows_x.append(fcX[k - 1] * coef)
        rows_y.append(fcY[k - 1])
        rows_x.append(fsX[k - 1] * coef)
        rows_y.append(fsY[k - 1])
    AX = np.concatenate(rows_x, axis=0)  # [KO, S]
    AY = np.concatenate(rows_y, axis=0)

    FX = np.zeros((KPAD, S), np.float32)
    FY = np.zeros((KPAD, S), np.float32)
    FX[:KC] = -0.5 * xn
    FY[:KC] = yn
    FX[KC:KC + KO] = -AX / 98.0
    FY[KC:KC + KO] = AY
    FX[KC + KO] = 1.0
    FY[KC + KO] = 0.5 + 0.5 * P2 / 98.0
    FX[KC + KO + 1] = 0.5 * R2 / 98.0
    FY[KC + KO + 1] = 1.0
    return FX, FY


def _cascade_bias(FX, FY):
    """Replace the 2 bf16 bias rows with 4 fp8 residual-cascade rows."""
    import ml_dtypes as _mld

    def q8(a):
        return (a * 16.0).astype(_mld.float8_e4m3).astype(np.float32) / 16.0

    FXz = FX.copy()
    FYz = FY.copy()
    FXz[KC + KO:] = 0.0
    FYz[KC + KO:] = 0.0
    by = FY[KC + KO]       # 0.5 + 0.5*P2/98 (paired with x=1)
    bx = FX[KC + KO + 1]   # 0.5*R2/98      (paired with y=1)
    q1 = q8(by)
    FXz[KC + KO] = 1.0
    FYz[KC + KO] = q1
    FXz[KC + KO + 1] = 1.0
    FYz[KC + KO + 1] = by - q1
    q2 = q8(bx)
    FXz[KC + KO + 2] = q2
    FYz[KC + KO + 2] = 1.0
    FXz[KC + KO + 3] = bx - q2
    FYz[KC + KO + 3] = 1.0
    return FXz, FYz


def _build_bass(stage=6):
    import concourse.bass as bass
    from concourse import mybir, bacc
    from concourse.tile import TileContext

    f32 = mybir.dt.float32
    f32r = mybir.dt.float32r
    bf16 = mybir.dt.bfloat16
    u32 = mybir.dt.uint32
    Alu = mybir.AluOpType
    Act = mybir.ActivationFunctionType

    fp8 = mybir.dt.float8e4
    nc = bacc.Bacc("TRN2", target_bir_lowering=False, debug=False,
                   num_devices=NCORES)
    xw_ext = nc.declare_dram_parameter("xw", [128, KT * 512], fp8, isOutput=False)
    yw_ext = nc.declare_dram_parameter("yw", [NCH, 128, KT * 512], fp8, isOutput=False)
    sout_ext = nc.declare_dram_parameter("sout", [128, 8], f32, isOutput=True)

    with TileContext(nc) as tc:
        with tc.tile_pool(name="xp", bufs=1) as xp, \
             tc.tile_pool(name="yp", bufs=3) as yp, \
             tc.tile_pool(name="dp", bufs=1) as dp, \
             tc.tile_pool(name="sp", bufs=2) as sp, \
             tc.tile_pool(name="hp", bufs=2) as hp, \
             tc.tile_pool(name="pp", bufs=7, space="PSUM") as pp, \
             tc.tile_pool(name="cp", bufs=1, space="PSUM") as cp, \
             tc.tile_pool(name="dr", bufs=2, space="DRAM") as dr:

            xw = xp.tile([128, KT, 512], fp8)
            xv = xw_ext.rearrange("p (a b) -> p a b", a=KT)
            nc.sync.dma_start(out=xw[:, 0:KT // 2, :], in_=xv[:, 0:KT // 2, :])
            nc.gpsimd.dma_start(out=xw[:, KT // 2:KT, :], in_=xv[:, KT // 2:KT, :])

            D = [dp.tile([128, S], bf16, tag=f"D{m}", name=f"D{m}") for m in range(MT)]
            mnc = sp.tile([128, MT * NCH], f32, tag="mnc", bufs=1)   # chunk mins
            sout = sp.tile([128, 8], f32, tag="sout", bufs=1)
            mlo = sp.tile([128, MT], f32, tag="mlo", bufs=1)
            mhi = sp.tile([128, MT], f32, tag="mhi", bufs=1)
            mlob = sp.tile([128, MT], bf16, tag="mlob", bufs=1)
            mhib = sp.tile([128, MT], bf16, tag="mhib", bufs=1)
            ilo = sp.tile([128, MT], mybir.dt.uint16, tag="ilo", bufs=1)
            ihi = sp.tile([128, MT], mybir.dt.uint16, tag="ihi", bufs=1)
            amu = sp.tile([128, MT], mybir.dt.uint16, tag="amu", bufs=1)

            # ---- Phase 1: fused GEMM dA = FX.T @ FY (fp8 DoubleRow + bf16 bias) ----
            KQ = KT // 4
            for n in range(NCH):
                y = yp.tile([128, KT, 512], fp8, tag="y")
                yv = yw_ext[n].rearrange("p (a b) -> p a b", a=KT)
                for qi, eng in ((0, nc.sync), (1, nc.gpsimd), (2, nc.sync), (3, nc.gpsimd)):
                    eng.dma_start(out=y[:, qi * KQ:(qi + 1) * KQ, :],
                                  in_=yv[:, qi * KQ:(qi + 1) * KQ, :])
                for m in range(MT):
                    ps = pp.tile([128, 512], f32, tag="ps")
                    for k in range(KT // 2):
                        nc.tensor.matmul(
                            out=ps,
                            lhsT=xw[:, 2 * k:2 * k + 2, m * 128:(m + 1) * 128],
                            rhs=y[:, 2 * k:2 * k + 2, :],
                            start=(k == 0),
                            stop=(k == KT // 2 - 1),
                            perf_mode=mybir.MatmulPerfMode.DoubleRow,
                        )
                    nc.scalar.activation(
                        out=D[m][:, n * 512:(n + 1) * 512], in_=ps,
                        func=Act.Copy, bias=0.0, scale=1.0 / 256.0)
                    mn = m * NCH + n
                    nc.vector.tensor_reduce(
                        out=mnc[:, mn:mn + 1],
                        in_=D[m][:, n * 512:(n + 1) * 512],
                        axis=mybir.AxisListType.X, op=Alu.min)
                    if stage >= 2 and n == NCH // 2 - 1:
                        # low-half argmin scan: hides under GEMM of chunks 4-7
                        nc.vector.tensor_reduce(
                            out=mlo[:, m:m + 1], in_=mnc[:, m * NCH:m * NCH + 4],
                            axis=mybir.AxisListType.X, op=Alu.min)
                        nc.vector.tensor_copy(out=mlob[:, m:m + 1],
                                              in_=mlo[:, m:m + 1])
                        alo8 = hp.tile([128, 8], mybir.dt.uint16, tag="alo8")
                        nc.vector.max_index(
                            out=alo8,
                            in_max=mlob[:, m:m + 1].broadcast_to([128, 8]),
                            in_values=D[m][:, 0:S // 2])
                        nc.vector.tensor_copy(out=ilo[:, m:m + 1], in_=alo8[:, 0:1])
                    if stage >= 2 and n == NCH - 1:
                        nc.vector.tensor_reduce(
                            out=mhi[:, m:m + 1],
                            in_=mnc[:, m * NCH + 4:(m + 1) * NCH],
                            axis=mybir.AxisListType.X, op=Alu.min)
                        nc.vector.tensor_copy(out=mhib[:, m:m + 1],
                                              in_=mhi[:, m:m + 1])
                        ahi8 = hp.tile([128, 8], mybir.dt.uint16, tag="ahi8")
                        nc.vector.max_index(
                            out=ahi8,
                            in_max=mhib[:, m:m + 1].broadcast_to([128, 8]),
                            in_values=D[m][:, S // 2:S])
                        nc.vector.tensor_copy(out=ihi[:, m:m + 1], in_=ahi8[:, 0:1])

            if stage < 2:
                for m in range(MT):
                    nc.vector.tensor_reduce(
                        out=sout[:, m:m + 1], in_=mnc[:, m * NCH:(m + 1) * NCH],
                        axis=mybir.AxisListType.X, op=Alu.min)
                    nc.vector.tensor_copy(out=sout[:, 4 + m:5 + m],
                                          in_=mnc[:, m * NCH:m * NCH + 1])

            if stage >= 2:
                # combine half-row argmins (ties -> low half, numpy first-occurrence)
                ihig = sp.tile([128, MT], mybir.dt.uint16, tag="ihig", bufs=1)
                nc.vector.tensor_scalar(out=ihig, in0=ihi, scalar1=2048,
                                        scalar2=None, op0=Alu.add)
                lomask = sp.tile([128, MT], mybir.dt.int32, tag="lomask", bufs=1)
                nc.vector.tensor_tensor(out=lomask, in0=mlo, in1=mhi,
                                        op=Alu.is_le)
                nc.vector.select(out=amu, mask=lomask, on_true=ilo, on_false=ihig)

                # ---- Phase 2: allgather argmins, histogram ----
                am_loc = dr.tile([MSH], mybir.dt.uint16)
                nc.sync.dma_start(out=am_loc.rearrange("(m p) -> p m", m=MT),
                                  in_=amu)
                am_all = dr.tile([S], mybir.dt.uint16)
                nc.gpsimd.collective_compute(
                    "AllGather", Alu.bypass,
                    replica_groups=[list(range(NCORES))],
                    ins=[am_loc[:].opt()], outs=[am_all[:].opt()],
                )
                amsb = hp.tile([128, 32], mybir.dt.uint16, tag="amsb")
                nc.sync.dma_start(out=amsb, in_=am_all.rearrange("(p f) -> p f", p=128))

                hi = hp.tile([128, 32], mybir.dt.uint16, tag="hi")
                lo = hp.tile([128, 32], mybir.dt.uint16, tag="lo")
                nc.vector.tensor_scalar(out=hi, in0=amsb, scalar1=5, scalar2=None,
                                        op0=Alu.logical_shift_right)
                nc.vector.tensor_scalar(out=lo, in0=amsb, scalar1=31, scalar2=None,
                                        op0=Alu.bitwise_and)
                hif = hp.tile([128, 32], f32, tag="hif")
                lof = hp.tile([128, 32], f32, tag="lof")
                nc.vector.tensor_copy(out=hif, in_=hi)
                nc.vector.tensor_copy(out=lof, in_=lo)

                iota_a = hp.tile([128, 128], f32, tag="iota_a", bufs=1)
                nc.gpsimd.iota(iota_a, pattern=[[1, 128]], base=0,
                               channel_multiplier=0,
                               allow_small_or_imprecise_dtypes=True)
                iota_b = hp.tile([128, 32], f32, tag="iota_b", bufs=1)
                nc.gpsimd.iota(iota_b, pattern=[[1, 32]], base=0,
                               channel_multiplier=0,
                               allow_small_or_imprecise_dtypes=True)

                Hall = hp.tile([128, 32, 128], bf16, tag="Hall", bufs=1)
                Lall = hp.tile([128, 32, 32], bf16, tag="Lall", bufs=1)
                nc.vector.tensor_tensor(
                    out=Hall,
                    in0=hif.rearrange("p (f o) -> p f o", o=1)
                        .broadcast_to([128, 32, 128]),
                    in1=iota_a.rearrange("p (o a) -> p o a", o=1)
                        .broadcast_to([128, 32, 128]),
                    op=Alu.is_equal)
                nc.vector.tensor_tensor(
                    out=Lall,
                    in0=lof.rearrange("p (f o) -> p f o", o=1)
                        .broadcast_to([128, 32, 32]),
                    in1=iota_b.rearrange("p (o b) -> p o b", o=1)
                        .broadcast_to([128, 32, 32]),
                    op=Alu.is_equal)

                cnt_ps = cp.tile([128, 32], f32, tag="cnt")
                for f in range(32):
                    nc.tensor.matmul(out=cnt_ps, lhsT=Hall[:, f, :],
                                     rhs=Lall[:, f, :],
                                     start=(f == 0), stop=(f == 31))

            if stage == 2:
                csb2 = hp.tile([128, 32], f32, tag="csb2", bufs=1)
                nc.scalar.copy(out=csb2, in_=cnt_ps)
                nc.vector.tensor_copy(out=sout[:, 0:8], in_=csb2[:, 0:8])

            if stage >= 3:
                # counts [128a, 32b] (t = 32a + b) -> pen_row [1, 4096] f32
                csb = hp.tile([128, 32], f32, tag="csb", bufs=1)
                nc.scalar.copy(out=csb, in_=cnt_ps)
                cnt_dram = dr.tile([S], f32)
                nc.sync.dma_start(out=cnt_dram.rearrange("(p f) -> p f", p=128),
                                  in_=csb)
                pen_row = hp.tile([1, S], f32, tag="pen_row", bufs=1)
                nc.sync.dma_start(out=pen_row,
                                  in_=cnt_dram.rearrange("(o s) -> o s", o=1))
                w005 = hp.tile([1, 128], f32, tag="w005", bufs=1)
                nc.vector.memset(w005, OCC_W)
                # pen broadcast tiles [128, 4096] bf16
                pen_bc = sp.tile([128, S], bf16, tag="pen_bc", bufs=1)
                for n in range(NCH):
                    pen_ps = pp.tile([128, 512], f32, tag="ps")
                    nc.tensor.matmul(out=pen_ps,
                                     lhsT=w005.bitcast(f32r),
                                     rhs=pen_row[:, n * 512:(n + 1) * 512]
                                         .bitcast(f32r),
                                     start=True, stop=True,
                                     skip_group_check=True)
                    nc.scalar.copy(out=pen_bc[:, n * 512:(n + 1) * 512], in_=pen_ps)

            if stage >= 5:
                # ---- Phase 3: Dp = dA + pen, row min + exp row sums ----
                mfin = sp.tile([128, MT], f32, tag="mfin", bufs=1)
                it4 = sp.tile([128, MT], f32, tag="it4", bufs=1)
                nit4 = sp.tile([128, MT], f32, tag="nit4", bufs=1)
                bia4 = sp.tile([128, MT], f32, tag="bia4", bufs=1)
                half = sp.tile([128, MT], f32, tag="half", bufs=1)
                for m in range(MT):
                    nc.vector.tensor_tensor(out=D[m], in0=D[m], in1=pen_bc,
                                            op=Alu.add)
                    nc.vector.tensor_reduce(
                        out=mfin[:, m:m + 1], in_=D[m],
                        axis=mybir.AxisListType.X, op=Alu.min)
                    nc.vector.tensor_scalar(out=half[:, m:m + 1],
                                            in0=mfin[:, m:m + 1], scalar1=1e-5,
                                            scalar2=H_PARAM,
                                            op0=Alu.add, op1=Alu.mult)
                    nc.vector.reciprocal(out=it4[:, m:m + 1], in_=half[:, m:m + 1])
                    nc.vector.tensor_scalar(out=nit4[:, m:m + 1],
                                            in0=it4[:, m:m + 1], scalar1=-1.0,
                                            scalar2=None, op0=Alu.mult)
                    nc.vector.tensor_tensor(out=bia4[:, m:m + 1],
                                            in0=mfin[:, m:m + 1],
                                            in1=it4[:, m:m + 1], op=Alu.mult)
                    ed = sp.tile([128, S], bf16, tag="ed")
                    nc.scalar.activation(
                        out=ed, in_=D[m],
                        func=Act.Exp,
                        bias=bia4[:, m:m + 1], scale=nit4[:, m:m + 1],
                        accum_out=sout[:, m:m + 1])
                nc.vector.tensor_copy(out=sout[:, 4:4 + MT], in_=mfin)

            nc.sync.dma_start(out=sout_ext[:], in_=sout)

    nc.compile()
    return nc


def _pack_inputs(target_features, reference_features, target_orient, refer_orient,
                 target_field, refer_field):
    iy_t, ix_t = _grid_idx(np.asarray(target_field[0], dtype=np.float32))
    iy_r, ix_r = _grid_idx(np.asarray(refer_field[0], dtype=np.float32))

    tf = _gather_patches(np.asarray(target_features[0], np.float32), iy_t, ix_t)
    rf = _gather_patches(np.asarray(reference_features[0], np.float32), iy_r, ix_r)
    to = _gather_patches(np.asarray(target_orient[0], np.float32), iy_t, ix_t)
    ro = _gather_patches(np.asarray(refer_orient[0], np.float32), iy_r, ix_r)

    FX, FY = _host_features(tf, rf, to, ro)
    FXz, FYz = _cascade_bias(FX, FY)
    FXq = (FXz * 16.0).astype(ml_dtypes.float8_e4m3).reshape(KT, 128, S)
    FYq = (FYz * 16.0).astype(ml_dtypes.float8_e4m3).reshape(KT, 128, NCH, 512)
    yw = np.ascontiguousarray(FYq.transpose(2, 1, 0, 3)).reshape(NCH, 128, KT * 512)
    in_maps = []
    for c in range(NCORES):
        xw_c = np.ascontiguousarray(
            FXq[:, :, c * MSH:(c + 1) * MSH].transpose(1, 0, 2)
        ).reshape(128, KT * 512)
        in_maps.append({"xw": xw_c, "yw": yw})
    return in_maps, (FXz, FYz)


def kernel(target_features, reference_features, target_orient, refer_orient,
           target_field, refer_field):
    global DEVICE_OK, LAST_EXEC_NS
    in_maps, (FXz, FYz) = _pack_inputs(target_features, reference_features,
                                       target_orient, refer_orient,
                                       target_field, refer_field)
    try:
        from concourse.bass_utils import run_bass_kernel_spmd
        if "nc" not in _BASS_CACHE:
            _BASS_CACHE["nc"] = _build_bass()
        nc = _BASS_CACHE["nc"]
        res = run_bass_kernel_spmd(nc, in_maps, list(range(NCORES)))
        LAST_EXEC_NS = getattr(res, "exec_time_ns", None)
        S_all = np.empty(S, np.float32)
        for c in range(NCORES):
            o = res.results[c]["sout"]  # [128, 8]
            for m in range(MT):
                S_all[c * MSH + m * 128:c * MSH + (m + 1) * 128] = o[:, m]
        DEVICE_OK = True
        loss = np.log(S_all).mean()
        return np.float32(loss)
    except Exception:
        sys.stderr.write("device path failed; host fallback\n")
        import traceback
        traceback.print_exc()
        import ml_dtypes as _mld
        FXe = (FXz * 16.0).astype(_mld.float8_e4m3).astype(np.float32) / 16.0
        FYe = (FYz * 16.0).astype(_mld.float8_e4m3).astype(np.float32) / 16.0
        dA = FXe.T @ FYe
        am = np.argmin(dA, axis=1)
        counts = np.bincount(am, minlength=S).astype(np.float32)
        dtot = dA + OCC_W * counts[None, :]
        m = dtot.min(axis=1)
        it = 1.0 / (H_PARAM * (m + 1e-5))
        Srow = np.exp((m[:, None] - dtot) * it[:, None]).sum(axis=1)
        return np.float32(np.log(Srow).mean())


# revision 5
# speedup vs baseline: 1.1086x; 1.0043x over previous
import sys
import os
import numpy as np

for _p in ("/opt/trn_rl_repo",):
    if _p not in sys.path:
        sys.path.insert(0, _p)

import ml_dtypes

PATCH = 7
STRIDE = 3
GRID = 126  # (384 - 7)//3 + 1
SAMPLE = 64
S = SAMPLE * SAMPLE  # 4096
H_PARAM = 0.5
ORIENT_W = 0.5
OCC_W = 0.05
EPS_NORM = 1e-05

KF = 4                      # Fourier harmonics for |cos| expansion
KC = 3136                   # cosine feature rows
KO = 49 * (2 * KF + 1)      # 441 orientation feature rows
KPAD = 3584                 # 28 * 128
KT = KPAD // 128            # 28
NCORES = 8
MSH = S // NCORES           # 512 rows per core
NCH = 8                     # column chunks of 512
MT = MSH // 128             # 4 m-tiles per core

LAST_EXEC_NS = None
DEVICE_OK = False
_BASS_CACHE = {}


def _grid_idx(field):
    gx = field[..., 0].reshape(-1)
    gy = field[..., 1].reshape(-1)
    ix = np.clip(np.round((gx + 1.0) * GRID / 2.0 - 0.5).astype(np.int64), 0, GRID - 1)
    iy = np.clip(np.round((gy + 1.0) * GRID / 2.0 - 0.5).astype(np.int64), 0, GRID - 1)
    return iy, ix


def _gather_patches(feat, iy, ix):
    # feat [C, H, W] -> [C*49, S] with torch-unfold channel ordering (c*49 + ki*7+kj)
    C = feat.shape[0]
    by = iy * STRIDE
    bx = ix * STRIDE
    out = np.empty((C, PATCH * PATCH, S), dtype=np.float32)
    for ki in range(PATCH):
        for kj in range(PATCH):
            out[:, ki * PATCH + kj, :] = feat[:, by + ki, bx + kj]
    return out.reshape(C * PATCH * PATCH, S)


def _fourier_feats(o):
    # o [98, S]: 2 channels x 49 patch positions. Returns r [49,S], cos/sin harmonics.
    u = o.reshape(2, 49, S)[0]
    v = o.reshape(2, 49, S)[1]
    r2 = u * u + v * v
    r = np.sqrt(r2)
    safe = np.maximum(r2, 1e-30)
    c1 = (u * u - v * v) / safe
    s1 = 2.0 * u * v / safe
    fc, fs = [], []
    ck, sk = c1, s1
    for _k in range(1, KF + 1):
        fc.append(r * ck)
        fs.append(r * sk)
        ck, sk = ck * c1 - sk * s1, sk * c1 + ck * s1
    return r, fc, fs


def _host_features(tf, rf, to, ro):
    """Build FX, FY [KPAD, S] fp32 so that dA = FX.T @ FY."""
    ymean = rf.mean(axis=1, keepdims=True)
    xc = tf - ymean
    yc = rf - ymean
    xn = xc / (np.linalg.norm(xc, axis=0, keepdims=True) + EPS_NORM)
    yn = yc / (np.linalg.norm(yc, axis=0, keepdims=True) + EPS_NORM)

    xs = to.reshape(2, 49, S)
    ys = ro.reshape(2, 49, S)
    R2 = (xs * xs).sum(axis=0).sum(axis=0)  # [S]
    P2 = (ys * ys).sum(axis=0).sum(axis=0)

    rX, fcX, fsX = _fourier_feats(to)
    rY, fcY, fsY = _fourier_feats(ro)
    rows_x = [rX * (2.0 / np.pi)]
    rows_y = [rY]
    for k in range(1, KF + 1):
        coef = (4.0 / np.pi) * ((-1.0) ** (k + 1)) / (4.0 * k * k - 1.0)
        rows_x.append(fcX[k - 1] * coef)
        rows_y.append(fcY[k - 1])
        rows_x.append(fsX[k - 1] * coef)
        rows_y.append(fsY[k - 1])
    AX = np.concatenate(rows_x, axis=0)  # [KO, S]
    AY = np.concatenate(rows_y, axis=0)

    FX = np.zeros((KPAD, S), np.float32)
    FY = np.zeros((KPAD, S), np.float32)
    FX[:KC] = -0.5 * xn
    FY[:KC] = yn
    FX[KC:KC + KO] = -AX / 98.0
    FY[KC:KC + KO] = AY
    FX[KC + KO] = 1.0
    FY[KC + KO] = 0.5 + 0.5 * P2 / 98.0
    FX[KC + KO + 1] = 0.5 * R2 / 98.0
    FY[KC + KO + 1] = 1.0
    return FX, FY


def _cascade_bias(FX, FY):
    """Replace the 2 bf16 bias rows with 4 fp8 residual-cascade rows."""
    import ml_dtypes as _mld

    def q8(a):
        return (a * 16.0).astype(_mld.float8_e4m3).astype(np.float32) / 16.0

    FXz = FX.copy()
    FYz = FY.copy()
    FXz[KC + KO:] = 0.0
    FYz[KC + KO:] = 0.0
    by = FY[KC + KO]       # 0.5 + 0.5*P2/98 (paired with x=1)
    bx = FX[KC + KO + 1]   # 0.5*R2/98      (paired with y=1)
    q1 = q8(by)
    FXz[KC + KO] = 1.0
    FYz[KC + KO] = q1
    FXz[KC + KO + 1] = 1.0
    FYz[KC + KO + 1] = by - q1
    q2 = q8(bx)
    FXz[KC + KO + 2] = q2
    FYz[KC + KO + 2] = 1.0
    FXz[KC + KO + 3] = bx - q2
    FYz[KC + KO + 3] = 1.0
    return FXz, FYz


def _build_bass(stage=6):
    import concourse.bass as bass
    from concourse import mybir, bacc
    from concourse.tile import TileContext

    f32 = mybir.dt.float32
    f32r = mybir.dt.float32r
    bf16 = mybir.dt.bfloat16
    u32 = mybir.dt.uint32
    Alu = mybir.AluOpType
    Act = mybir.ActivationFunctionType

    fp8 = mybir.dt.float8e4
    nc = bacc.Bacc("TRN2", target_bir_lowering=False, debug=False,
                   num_devices=NCORES)
    xw_ext = nc.declare_dram_parameter("xw", [128, KT * 512], fp8, isOutput=False)
    yw_ext = nc.declare_dram_parameter("yw", [NCH, 128, KT * 512], fp8, isOutput=False)
    sout_ext = nc.declare_dram_parameter("sout", [128, 8], f32, isOutput=True)

    with TileContext(nc) as tc:
        with tc.tile_pool(name="xp", bufs=1) as xp, \
             tc.tile_pool(name="yp", bufs=3) as yp, \
             tc.tile_pool(name="dp", bufs=1) as dp, \
             tc.tile_pool(name="sp", bufs=2) as sp, \
             tc.tile_pool(name="hp", bufs=2) as hp, \
             tc.tile_pool(name="pp", bufs=7, space="PSUM") as pp, \
             tc.tile_pool(name="cp", bufs=1, space="PSUM") as cp, \
             tc.tile_pool(name="dr", bufs=2, space="DRAM") as dr:

            xw = xp.tile([128, KT, 512], fp8)
            xv = xw_ext.rearrange("p (a b) -> p a b", a=KT)
            nc.sync.dma_start(out=xw[:, 0:KT // 2, :], in_=xv[:, 0:KT // 2, :])
            nc.gpsimd.dma_start(out=xw[:, KT // 2:KT, :], in_=xv[:, KT // 2:KT, :])

            D = [dp.tile([128, S], bf16, tag=f"D{m}", name=f"D{m}") for m in range(MT)]
            mnc = sp.tile([128, MT * NCH], f32, tag="mnc", bufs=1)   # chunk mins
            sout = sp.tile([128, 8], f32, tag="sout", bufs=1)
            mlo = sp.tile([128, MT], f32, tag="mlo", bufs=1)
            mhi = sp.tile([128, MT], f32, tag="mhi", bufs=1)
            mlob = sp.tile([128, MT], bf16, tag="mlob", bufs=1)
            mhib = sp.tile([128, MT], bf16, tag="mhib", bufs=1)
            ilo = sp.tile([128, MT], mybir.dt.uint16, tag="ilo", bufs=1)
            ihi = sp.tile([128, MT], mybir.dt.uint16, tag="ihi", bufs=1)
            amu = sp.tile([128, MT], mybir.dt.uint16, tag="amu", bufs=1)

            # ---- Phase 1: fused GEMM dA = FX.T @ FY (fp8 DoubleRow + bf16 bias) ----
            KQ = KT // 4
            for n in range(NCH):
                y = yp.tile([128, KT, 512], fp8, tag="y")
                yv = yw_ext[n].rearrange("p (a b) -> p a b", a=KT)
                for qi, eng in ((0, nc.sync), (1, nc.gpsimd), (2, nc.sync), (3, nc.gpsimd)):
                    eng.dma_start(out=y[:, qi * KQ:(qi + 1) * KQ, :],
                                  in_=yv[:, qi * KQ:(qi + 1) * KQ, :])
                for m in range(MT):
                    ps = pp.tile([128, 512], f32, tag="ps")
                    for k in range(KT // 2):
                        nc.tensor.matmul(
                            out=ps,
                            lhsT=xw[:, 2 * k:2 * k + 2, m * 128:(m + 1) * 128],
                            rhs=y[:, 2 * k:2 * k + 2, :],
                            start=(k == 0),
                            stop=(k == KT // 2 - 1),
                            perf_mode=mybir.MatmulPerfMode.DoubleRow,
                        )
                    nc.scalar.activation(
                        out=D[m][:, n * 512:(n + 1) * 512], in_=ps,
                        func=Act.Copy, bias=0.0, scale=1.0 / 256.0)
                    mn = m * NCH + n
                    nc.vector.tensor_reduce(
                        out=mnc[:, mn:mn + 1],
                        in_=D[m][:, n * 512:(n + 1) * 512],
                        axis=mybir.AxisListType.X, op=Alu.min)
                    if stage >= 2 and n == NCH // 2 - 1:
                        # low-half argmin scan: hides under GEMM of chunks 4-7
                        nc.vector.tensor_reduce(
                            out=mlo[:, m:m + 1], in_=mnc[:, m * NCH:m * NCH + 4],
                            axis=mybir.AxisListType.X, op=Alu.min)
                        nc.vector.tensor_copy(out=mlob[:, m:m + 1],
                                              in_=mlo[:, m:m + 1])
                        alo8 = hp.tile([128, 8], mybir.dt.uint16, tag="alo8")
                        nc.vector.max_index(
                            out=alo8,
                            in_max=mlob[:, m:m + 1].broadcast_to([128, 8]),
                            in_values=D[m][:, 0:S // 2])
                        nc.vector.tensor_copy(out=ilo[:, m:m + 1], in_=alo8[:, 0:1])
                    if stage >= 2 and n == NCH - 1:
                        nc.vector.tensor_reduce(
                            out=mhi[:, m:m + 1],
                            in_=mnc[:, m * NCH + 4:(m + 1) * NCH],
                            axis=mybir.AxisListType.X, op=Alu.min)
                        nc.vector.tensor_copy(out=mhib[:, m:m + 1],
                                              in_=mhi[:, m:m + 1])
                        ahi8 = hp.tile([128, 8], mybir.dt.uint16, tag="ahi8")
                        nc.vector.max_index(
                            out=ahi8,
                            in_max=mhib[:, m:m + 1].broadcast_to([128, 8]),
                            in_values=D[m][:, S // 2:S])
                        nc.vector.tensor_copy(out=ihi[:, m:m + 1], in_=ahi8[:, 0:1])

            if stage < 2:
                for m in range(MT):
                    nc.vector.tensor_reduce(
                        out=sout[:, m:m + 1], in_=mnc[:, m * NCH:(m + 1) * NCH],
                        axis=mybir.AxisListType.X, op=Alu.min)
                    nc.vector.tensor_copy(out=sout[:, 4 + m:5 + m],
                                          in_=mnc[:, m * NCH:m * NCH + 1])

            if stage >= 2:
                # combine half-row argmins (ties -> low half, numpy first-occurrence)
                ihig = sp.tile([128, MT], mybir.dt.uint16, tag="ihig", bufs=1)
                nc.vector.tensor_scalar(out=ihig, in0=ihi, scalar1=2048,
                                        scalar2=None, op0=Alu.add)
                lomask = sp.tile([128, MT], mybir.dt.int32, tag="lomask", bufs=1)
                nc.vector.tensor_tensor(out=lomask, in0=mlo, in1=mhi,
                                        op=Alu.is_le)
                nc.vector.select(out=amu, mask=lomask, on_true=ilo, on_false=ihig)

                # ---- Phase 2: allgather argmins, histogram ----
                am_loc = dr.tile([MSH], mybir.dt.uint16)
                nc.sync.dma_start(out=am_loc.rearrange("(m p) -> p m", m=MT),
                                  in_=amu)
                am_all = dr.tile([S], mybir.dt.uint16)
                nc.gpsimd.collective_compute(
                    "AllGather", Alu.bypass,
                    replica_groups=[list(range(NCORES))],
                    ins=[am_loc[:].opt()], outs=[am_all[:].opt()],
                )
                amsb = hp.tile([128, 32], mybir.dt.uint16, tag="amsb")
                nc.sync.dma_start(out=amsb, in_=am_all.rearrange("(p f) -> p f", p=128))

                hi = hp.tile([128, 32], mybir.dt.uint16, tag="hi")
                lo = hp.tile([128, 32], mybir.dt.uint16, tag="lo")
                nc.vector.tensor_scalar(out=hi, in0=amsb, scalar1=5, scalar2=None,
                                        op0=Alu.logical_shift_right)
                nc.vector.tensor_scalar(out=lo, in0=amsb, scalar1=31, scalar2=None,
                                        op0=Alu.bitwise_and)
                hif = hp.tile([128, 32], f32, tag="hif")
                lof = hp.tile([128, 32], f32, tag="lof")
                nc.vector.tensor_copy(out=hif, in_=hi)
                nc.vector.tensor_copy(out=lof, in_=lo)

                iota_a = hp.tile([128, 128], f32, tag="iota_a", bufs=1)
                nc.gpsimd.iota(iota_a, pattern=[[1, 128]], base=0,
                               channel_multiplier=0,
                               allow_small_or_imprecise_dtypes=True)
                iota_b = hp.tile([128, 32], f32, tag="iota_b", bufs=1)
                nc.gpsimd.iota(iota_b, pattern=[[1, 32]], base=0,
                               channel_multiplier=0,
                               allow_small_or_imprecise_dtypes=True)

                Hall = hp.tile([128, 32, 128], fp8, tag="Hall", bufs=1)
                Lall = hp.tile([128, 32, 32], fp8, tag="Lall", bufs=1)
                nc.vector.tensor_tensor(
                    out=Hall,
                    in0=hif.rearrange("p (f o) -> p f o", o=1)
                        .broadcast_to([128, 32, 128]),
                    in1=iota_a.rearrange("p (o a) -> p o a", o=1)
                        .broadcast_to([128, 32, 128]),
                    op=Alu.is_equal)
                nc.vector.tensor_tensor(
                    out=Lall,
                    in0=lof.rearrange("p (f o) -> p f o", o=1)
                        .broadcast_to([128, 32, 32]),
                    in1=iota_b.rearrange("p (o b) -> p o b", o=1)
                        .broadcast_to([128, 32, 32]),
                    op=Alu.is_equal)

                cnt_ps = cp.tile([128, 32], f32, tag="cnt")
                for f in range(16):
                    nc.tensor.matmul(out=cnt_ps,
                                     lhsT=Hall[:, 2 * f:2 * f + 2, :],
                                     rhs=Lall[:, 2 * f:2 * f + 2, :],
                                     start=(f == 0), stop=(f == 15),
                                     perf_mode=mybir.MatmulPerfMode.DoubleRow)

            if stage == 2:
                csb2 = hp.tile([128, 32], f32, tag="csb2", bufs=1)
                nc.scalar.copy(out=csb2, in_=cnt_ps)
                nc.vector.tensor_copy(out=sout[:, 0:8], in_=csb2[:, 0:8])

            if stage >= 3:
                # counts [128a, 32b] (t = 32a + b) -> pen_row [1, 4096] f32
                csb = hp.tile([128, 32], f32, tag="csb", bufs=1)
                nc.scalar.copy(out=csb, in_=cnt_ps)
                cnt_dram = dr.tile([S], f32)
                nc.sync.dma_start(out=cnt_dram.rearrange("(p f) -> p f", p=128),
                                  in_=csb)
                pen_row = hp.tile([1, S], f32, tag="pen_row", bufs=1)
                nc.sync.dma_start(out=pen_row,
                                  in_=cnt_dram.rearrange("(o s) -> o s", o=1))
                w005 = hp.tile([1, 128], f32, tag="w005", bufs=1)
                nc.vector.memset(w005, OCC_W)
                # pen broadcast tiles [128, 4096] bf16
                pen_bc = sp.tile([128, S], bf16, tag="pen_bc", bufs=1)
                for n in range(NCH):
                    pen_ps = pp.tile([128, 512], f32, tag="ps")
                    nc.tensor.matmul(out=pen_ps,
                                     lhsT=w005.bitcast(f32r),
                                     rhs=pen_row[:, n * 512:(n + 1) * 512]
                                         .bitcast(f32r),
                                     start=True, stop=True,
                                     skip_group_check=True)
                    nc.scalar.copy(out=pen_bc[:, n * 512:(n + 1) * 512], in_=pen_ps)

            if stage >= 5:
                # ---- Phase 3: Dp = dA + pen, row min + exp row sums ----
                mfin = sp.tile([128, MT], f32, tag="mfin", bufs=1)
                it4 = sp.tile([128, MT], f32, tag="it4", bufs=1)
                nit4 = sp.tile([128, MT], f32, tag="nit4", bufs=1)
                bia4 = sp.tile([128, MT], f32, tag="bia4", bufs=1)
                half = sp.tile([128, MT], f32, tag="half", bufs=1)
                for m in range(MT):
                    nc.vector.tensor_tensor(out=D[m], in0=D[m], in1=pen_bc,
                                            op=Alu.add)
                    nc.vector.tensor_reduce(
                        out=mfin[:, m:m + 1], in_=D[m],
                        axis=mybir.AxisListType.X, op=Alu.min)
                    nc.vector.tensor_scalar(out=half[:, m:m + 1],
                                            in0=mfin[:, m:m + 1], scalar1=1e-5,
                                            scalar2=H_PARAM,
                                            op0=Alu.add, op1=Alu.mult)
                    nc.vector.reciprocal(out=it4[:, m:m + 1], in_=half[:, m:m + 1])
                    nc.vector.tensor_scalar(out=nit4[:, m:m + 1],
                                            in0=it4[:, m:m + 1], scalar1=-1.0,
                                            scalar2=None, op0=Alu.mult)
                    nc.vector.tensor_tensor(out=bia4[:, m:m + 1],
                                            in0=mfin[:, m:m + 1],
                                            in1=it4[:, m:m + 1], op=Alu.mult)
                    ed = sp.tile([128, S], bf16, tag="ed")
                    nc.scalar.activation(
                        out=ed, in_=D[m],
                        func=Act.Exp,
                        bias=bia4[:, m:m + 1], scale=nit4[:, m:m + 1],
                        accum_out=sout[:, m:m + 1])
                nc.vector.tensor_copy(out=sout[:, 4:4 + MT], in_=mfin)

            nc.sync.dma_start(out=sout_ext[:], in_=sout)

    nc.compile()
    return nc


def _pack_inputs(target_features, reference_features, target_orient, refer_orient,
                 target_field, refer_field):
    iy_t, ix_t = _grid_idx(np.asarray(target_field[0], dtype=np.float32))
    iy_r, ix_r = _grid_idx(np.asarray(refer_field[0], dtype=np.float32))

    tf = _gather_patches(np.asarray(target_features[0], np.float32), iy_t, ix_t)
    rf = _gather_patches(np.asarray(reference_features[0], np.float32), iy_r, ix_r)
    to = _gather_patches(np.asarray(target_orient[0], np.float32), iy_t, ix_t)
    ro = _gather_patches(np.asarray(refer_orient[0], np.float32), iy_r, ix_r)

    FX, FY = _host_features(tf, rf, to, ro)
    FXz, FYz = _cascade_bias(FX, FY)
    FXq = (FXz * 16.0).astype(ml_dtypes.float8_e4m3).reshape(KT, 128, S)
    FYq = (FYz * 16.0).astype(ml_dtypes.float8_e4m3).reshape(KT, 128, NCH, 512)
    yw = np.ascontiguousarray(FYq.transpose(2, 1, 0, 3)).reshape(NCH, 128, KT * 512)
    in_maps = []
    for c in range(NCORES):
        xw_c = np.ascontiguousarray(
            FXq[:, :, c * MSH:(c + 1) * MSH].transpose(1, 0, 2)
        ).reshape(128, KT * 512)
        in_maps.append({"xw": xw_c, "yw": yw})
    return in_maps, (FXz, FYz)


def kernel(target_features, reference_features, target_orient, refer_orient,
           target_field, refer_field):
    global DEVICE_OK, LAST_EXEC_NS
    in_maps, (FXz, FYz) = _pack_inputs(target_features, reference_features,
                                       target_orient, refer_orient,
                                       target_field, refer_field)
    try:
        from concourse.bass_utils import run_bass_kernel_spmd
        if "nc" not in _BASS_CACHE:
            _BASS_CACHE["nc"] = _build_bass()
        nc = _BASS_CACHE["nc"]
        res = run_bass_kernel_spmd(nc, in_maps, list(range(NCORES)))
        LAST_EXEC_NS = getattr(res, "exec_time_ns", None)
        S_all = np.empty(S, np.float32)
        for c in range(NCORES):
            o = res.results[c]["sout"]  # [128, 8]
            for m in range(MT):
                S_all[c * MSH + m * 128:c * MSH + (m + 1) * 128] = o[:, m]
        DEVICE_OK = True
        loss = np.log(S_all).mean()
        return np.float32(loss)
    except Exception:
        sys.stderr.write("device path failed; host fallback\n")
        import traceback
        traceback.print_exc()
        import ml_dtypes as _mld
        FXe = (FXz * 16.0).astype(_mld.float8_e4m3).astype(np.float32) / 16.0
        FYe = (FYz * 16.0).astype(_mld.float8_e4m3).astype(np.float32) / 16.0
        dA = FXe.T @ FYe
        am = np.argmin(dA, axis=1)
        counts = np.bincount(am, minlength=S).astype(np.float32)
        dtot = dA + OCC_W * counts[None, :]
        m = dtot.min(axis=1)
        it = 1.0 / (H_PARAM * (m + 1e-5))
        Srow = np.exp((m[:, None] - dtot) * it[:, None]).sum(axis=1)
        return np.float32(np.log(Srow).mean())


# revision 6
# speedup vs baseline: 1.1314x; 1.0205x over previous
import sys
import os
import numpy as np

for _p in ("/opt/trn_rl_repo",):
    if _p not in sys.path:
        sys.path.insert(0, _p)

import ml_dtypes

PATCH = 7
STRIDE = 3
GRID = 126  # (384 - 7)//3 + 1
SAMPLE = 64
S = SAMPLE * SAMPLE  # 4096
H_PARAM = 0.5
ORIENT_W = 0.5
OCC_W = 0.05
EPS_NORM = 1e-05

KF = 4                      # Fourier harmonics for |cos| expansion
KC = 3136                   # cosine feature rows
KO = 49 * (2 * KF + 1)      # 441 orientation feature rows
KPAD = 3584                 # 28 * 128
KT = KPAD // 128            # 28
NCORES = 8
MSH = S // NCORES           # 512 rows per core
NCH = 8                     # column chunks of 512
MT = MSH // 128             # 4 m-tiles per core

LAST_EXEC_NS = None
DEVICE_OK = False
_BASS_CACHE = {}


def _grid_idx(field):
    gx = field[..., 0].reshape(-1)
    gy = field[..., 1].reshape(-1)
    ix = np.clip(np.round((gx + 1.0) * GRID / 2.0 - 0.5).astype(np.int64), 0, GRID - 1)
    iy = np.clip(np.round((gy + 1.0) * GRID / 2.0 - 0.5).astype(np.int64), 0, GRID - 1)
    return iy, ix


def _gather_patches(feat, iy, ix):
    # feat [C, H, W] -> [C*49, S] with torch-unfold channel ordering (c*49 + ki*7+kj)
    C = feat.shape[0]
    by = iy * STRIDE
    bx = ix * STRIDE
    out = np.empty((C, PATCH * PATCH, S), dtype=np.float32)
    for ki in range(PATCH):
        for kj in range(PATCH):
            out[:, ki * PATCH + kj, :] = feat[:, by + ki, bx + kj]
    return out.reshape(C * PATCH * PATCH, S)


def _fourier_feats(o):
    # o [98, S]: 2 channels x 49 patch positions. Returns r [49,S], cos/sin harmonics.
    u = o.reshape(2, 49, S)[0]
    v = o.reshape(2, 49, S)[1]
    r2 = u * u + v * v
    r = np.sqrt(r2)
    safe = np.maximum(r2, 1e-30)
    c1 = (u * u - v * v) / safe
    s1 = 2.0 * u * v / safe
    fc, fs = [], []
    ck, sk = c1, s1
    for _k in range(1, KF + 1):
        fc.append(r * ck)
        fs.append(r * sk)
        ck, sk = ck * c1 - sk * s1, sk * c1 + ck * s1
    return r, fc, fs


def _host_features(tf, rf, to, ro):
    """Build FX, FY [KPAD, S] fp32 so that dA = FX.T @ FY."""
    ymean = rf.mean(axis=1, keepdims=True)
    xc = tf - ymean
    yc = rf - ymean
    xn = xc / (np.linalg.norm(xc, axis=0, keepdims=True) + EPS_NORM)
    yn = yc / (np.linalg.norm(yc, axis=0, keepdims=True) + EPS_NORM)

    xs = to.reshape(2, 49, S)
    ys = ro.reshape(2, 49, S)
    R2 = (xs * xs).sum(axis=0).sum(axis=0)  # [S]
    P2 = (ys * ys).sum(axis=0).sum(axis=0)

    rX, fcX, fsX = _fourier_feats(to)
    rY, fcY, fsY = _fourier_feats(ro)
    rows_x = [rX * (2.0 / np.pi)]
    rows_y = [rY]
    for k in range(1, KF + 1):
        coef = (4.0 / np.pi) * ((-1.0) ** (k + 1)) / (4.0 * k * k - 1.0)
        rows_x.append(fcX[k - 1] * coef)
        rows_y.append(fcY[k - 1])
        rows_x.append(fsX[k - 1] * coef)
        rows_y.append(fsY[k - 1])
    AX = np.concatenate(rows_x, axis=0)  # [KO, S]
    AY = np.concatenate(rows_y, axis=0)

    FX = np.zeros((KPAD, S), np.float32)
    FY = np.zeros((KPAD, S), np.float32)
    FX[:KC] = -0.5 * xn
    FY[:KC] = yn
    FX[KC:KC + KO] = -AX / 98.0
    FY[KC:KC + KO] = AY
    FX[KC + KO] = 1.0
    FY[KC + KO] = 0.5 + 0.5 * P2 / 98.0
    FX[KC + KO + 1] = 0.5 * R2 / 98.0
    FY[KC + KO + 1] = 1.0
    return FX, FY


def _cascade_bias(FX, FY):
    """Replace the 2 bf16 bias rows with 4 fp8 residual-cascade rows."""
    import ml_dtypes as _mld

    def q8(a):
        return (a * 16.0).astype(_mld.float8_e4m3).astype(np.float32) / 16.0

    FXz = FX.copy()
    FYz = FY.copy()
    FXz[KC + KO:] = 0.0
    FYz[KC + KO:] = 0.0
    by = FY[KC + KO]       # 0.5 + 0.5*P2/98 (paired with x=1)
    bx = FX[KC + KO + 1]   # 0.5*R2/98      (paired with y=1)
    q1 = q8(by)
    FXz[KC + KO] = 1.0
    FYz[KC + KO] = q1
    FXz[KC + KO + 1] = 1.0
    FYz[KC + KO + 1] = by - q1
    q2 = q8(bx)
    FXz[KC + KO + 2] = q2
    FYz[KC + KO + 2] = 1.0
    FXz[KC + KO + 3] = bx - q2
    FYz[KC + KO + 3] = 1.0
    return FXz, FYz


def _build_bass(stage=6):
    import concourse.bass as bass
    from concourse import mybir, bacc
    from concourse.tile import TileContext

    f32 = mybir.dt.float32
    f32r = mybir.dt.float32r
    bf16 = mybir.dt.bfloat16
    u32 = mybir.dt.uint32
    Alu = mybir.AluOpType
    Act = mybir.ActivationFunctionType

    fp8 = mybir.dt.float8e4
    nc = bacc.Bacc("TRN2", target_bir_lowering=False, debug=False,
                   num_devices=NCORES)
    xw_ext = nc.declare_dram_parameter("xw", [128, KT * 512], fp8, isOutput=False)
    yw_ext = nc.declare_dram_parameter("yw", [NCH, 128, KT * 512], fp8, isOutput=False)
    sout_ext = nc.declare_dram_parameter("sout", [128, 8], f32, isOutput=True)

    with TileContext(nc) as tc:
        with tc.tile_pool(name="xp", bufs=1) as xp, \
             tc.tile_pool(name="yp", bufs=3) as yp, \
             tc.tile_pool(name="dp", bufs=1) as dp, \
             tc.tile_pool(name="sp", bufs=2) as sp, \
             tc.tile_pool(name="hp", bufs=2) as hp, \
             tc.tile_pool(name="pp", bufs=7, space="PSUM") as pp, \
             tc.tile_pool(name="cp", bufs=1, space="PSUM") as cp, \
             tc.tile_pool(name="dr", bufs=2, space="DRAM") as dr:

            xw = xp.tile([128, KT, 512], fp8)
            xv = xw_ext.rearrange("p (a b) -> p a b", a=KT)
            nc.sync.dma_start(out=xw[:, 0:KT // 2, :], in_=xv[:, 0:KT // 2, :])
            nc.gpsimd.dma_start(out=xw[:, KT // 2:KT, :], in_=xv[:, KT // 2:KT, :])

            D = [dp.tile([128, S], bf16, tag=f"D{m}", name=f"D{m}") for m in range(MT)]
            mnc = sp.tile([128, MT * NCH], f32, tag="mnc", bufs=1)   # chunk mins
            sout = sp.tile([128, 8], f32, tag="sout", bufs=1)
            mlo = sp.tile([128, MT], f32, tag="mlo", bufs=1)
            mhi = sp.tile([128, MT], f32, tag="mhi", bufs=1)
            mlob = sp.tile([128, MT], bf16, tag="mlob", bufs=1)
            mhib = sp.tile([128, MT], bf16, tag="mhib", bufs=1)
            ilo = sp.tile([128, MT], mybir.dt.uint16, tag="ilo", bufs=1)
            ihi = sp.tile([128, MT], mybir.dt.uint16, tag="ihi", bufs=1)
            amu = sp.tile([128, MT], mybir.dt.uint16, tag="amu", bufs=1)

            # ---- Phase 1: fused GEMM dA = FX.T @ FY (fp8 DoubleRow + bf16 bias) ----
            KQ = KT // 4
            for n in range(NCH):
                y = yp.tile([128, KT, 512], fp8, tag="y")
                yv = yw_ext[n].rearrange("p (a b) -> p a b", a=KT)
                for qi, eng in ((0, nc.sync), (1, nc.gpsimd), (2, nc.sync), (3, nc.gpsimd)):
                    eng.dma_start(out=y[:, qi * KQ:(qi + 1) * KQ, :],
                                  in_=yv[:, qi * KQ:(qi + 1) * KQ, :])
                for m in range(MT):
                    ps = pp.tile([128, 512], f32, tag="ps")
                    for k in range(KT // 2):
                        nc.tensor.matmul(
                            out=ps,
                            lhsT=xw[:, 2 * k:2 * k + 2, m * 128:(m + 1) * 128],
                            rhs=y[:, 2 * k:2 * k + 2, :],
                            start=(k == 0),
                            stop=(k == KT // 2 - 1),
                            perf_mode=mybir.MatmulPerfMode.DoubleRow,
                        )
                    nc.scalar.activation(
                        out=D[m][:, n * 512:(n + 1) * 512], in_=ps,
                        func=Act.Copy, bias=0.0, scale=1.0 / 256.0)
                    mn = m * NCH + n
                    nc.vector.tensor_reduce(
                        out=mnc[:, mn:mn + 1],
                        in_=D[m][:, n * 512:(n + 1) * 512],
                        axis=mybir.AxisListType.X, op=Alu.min)
                    if stage >= 2 and n == NCH // 2 - 1:
                        # low-half argmin scan: hides under GEMM of chunks 4-7
                        nc.vector.tensor_reduce(
                            out=mlo[:, m:m + 1], in_=mnc[:, m * NCH:m * NCH + 4],
                            axis=mybir.AxisListType.X, op=Alu.min)
                        nc.vector.tensor_copy(out=mlob[:, m:m + 1],
                                              in_=mlo[:, m:m + 1])
                        alo8 = hp.tile([128, 8], mybir.dt.uint16, tag="alo8")
                        nc.vector.max_index(
                            out=alo8,
                            in_max=mlob[:, m:m + 1].broadcast_to([128, 8]),
                            in_values=D[m][:, 0:S // 2])
                        nc.vector.tensor_copy(out=ilo[:, m:m + 1], in_=alo8[:, 0:1])
                    if stage >= 2 and n == NCH - 1:
                        nc.vector.tensor_reduce(
                            out=mhi[:, m:m + 1],
                            in_=mnc[:, m * NCH + 4:(m + 1) * NCH],
                            axis=mybir.AxisListType.X, op=Alu.min)
                        nc.vector.tensor_copy(out=mhib[:, m:m + 1],
                                              in_=mhi[:, m:m + 1])
                        ahi8 = hp.tile([128, 8], mybir.dt.uint16, tag="ahi8")
                        nc.vector.max_index(
                            out=ahi8,
                            in_max=mhib[:, m:m + 1].broadcast_to([128, 8]),
                            in_values=D[m][:, S // 2:S])
                        nc.vector.tensor_copy(out=ihi[:, m:m + 1], in_=ahi8[:, 0:1])

            if stage < 2:
                for m in range(MT):
                    nc.vector.tensor_reduce(
                        out=sout[:, m:m + 1], in_=mnc[:, m * NCH:(m + 1) * NCH],
                        axis=mybir.AxisListType.X, op=Alu.min)
                    nc.vector.tensor_copy(out=sout[:, 4 + m:5 + m],
                                          in_=mnc[:, m * NCH:m * NCH + 1])

            if stage >= 2:
                # combine half-row argmins (ties -> low half, numpy first-occurrence)
                ihig = sp.tile([128, MT], mybir.dt.uint16, tag="ihig", bufs=1)
                nc.vector.tensor_scalar(out=ihig, in0=ihi, scalar1=2048,
                                        scalar2=None, op0=Alu.add)
                lomask = sp.tile([128, MT], mybir.dt.int32, tag="lomask", bufs=1)
                nc.vector.tensor_tensor(out=lomask, in0=mlo, in1=mhi,
                                        op=Alu.is_le)
                nc.vector.select(out=amu, mask=lomask, on_true=ilo, on_false=ihig)

                # ---- Phase 2: allgather argmins, histogram ----
                am_loc = dr.tile([MSH], mybir.dt.uint16)
                nc.sync.dma_start(out=am_loc.rearrange("(m p) -> p m", m=MT),
                                  in_=amu)
                am_all = dr.tile([S], mybir.dt.uint16)
                nc.gpsimd.collective_compute(
                    "AllGather", Alu.bypass,
                    replica_groups=[list(range(NCORES))],
                    ins=[am_loc[:].opt()], outs=[am_all[:].opt()],
                )
                amsb = hp.tile([128, 32], mybir.dt.uint16, tag="amsb")
                nc.sync.dma_start(out=amsb, in_=am_all.rearrange("(p f) -> p f", p=128))

                hi = hp.tile([128, 32], mybir.dt.uint16, tag="hi")
                lo = hp.tile([128, 32], mybir.dt.uint16, tag="lo")
                nc.vector.tensor_scalar(out=hi, in0=amsb, scalar1=5, scalar2=None,
                                        op0=Alu.logical_shift_right)
                nc.vector.tensor_scalar(out=lo, in0=amsb, scalar1=31, scalar2=None,
                                        op0=Alu.bitwise_and)
                hif = hp.tile([128, 32], f32, tag="hif")
                lof = hp.tile([128, 32], f32, tag="lof")
                nc.vector.tensor_copy(out=hif, in_=hi)
                nc.vector.tensor_copy(out=lof, in_=lo)

                iota_a = hp.tile([128, 128], f32, tag="iota_a", bufs=1)
                nc.gpsimd.iota(iota_a, pattern=[[1, 128]], base=0,
                               channel_multiplier=0,
                               allow_small_or_imprecise_dtypes=True)
                iota_b = hp.tile([128, 32], f32, tag="iota_b", bufs=1)
                nc.gpsimd.iota(iota_b, pattern=[[1, 32]], base=0,
                               channel_multiplier=0,
                               allow_small_or_imprecise_dtypes=True)

                Hall = hp.tile([128, 32, 128], fp8, tag="Hall", bufs=1)
                Lall = hp.tile([128, 32, 32], fp8, tag="Lall", bufs=1)
                nc.vector.tensor_tensor(
                    out=Hall,
                    in0=hif.rearrange("p (f o) -> p f o", o=1)
                        .broadcast_to([128, 32, 128]),
                    in1=iota_a.rearrange("p (o a) -> p o a", o=1)
                        .broadcast_to([128, 32, 128]),
                    op=Alu.is_equal)
                nc.vector.tensor_tensor(
                    out=Lall,
                    in0=lof.rearrange("p (f o) -> p f o", o=1)
                        .broadcast_to([128, 32, 32]),
                    in1=iota_b.rearrange("p (o b) -> p o b", o=1)
                        .broadcast_to([128, 32, 32]),
                    op=Alu.is_equal)

                cnt_ps = cp.tile([128, 32], f32, tag="cnt")
                for f in range(16):
                    nc.tensor.matmul(out=cnt_ps,
                                     lhsT=Hall[:, 2 * f:2 * f + 2, :],
                                     rhs=Lall[:, 2 * f:2 * f + 2, :],
                                     start=(f == 0), stop=(f == 15),
                                     perf_mode=mybir.MatmulPerfMode.DoubleRow)

            if stage == 2:
                csb2 = hp.tile([128, 32], f32, tag="csb2", bufs=1)
                nc.scalar.copy(out=csb2, in_=cnt_ps)
                nc.vector.tensor_copy(out=sout[:, 0:8], in_=csb2[:, 0:8])

            if stage >= 3:
                # counts [128a, 32b] (t = 32a + b) -> pen_row [1, 4096] f32
                csb = hp.tile([128, 32], f32, tag="csb", bufs=1)
                nc.scalar.copy(out=csb, in_=cnt_ps)
                cnt_dram = dr.tile([S], f32)
                nc.sync.dma_start(out=cnt_dram.rearrange("(p f) -> p f", p=128),
                                  in_=csb)
                pen_row = hp.tile([1, S], f32, tag="pen_row", bufs=1)
                nc.sync.dma_start(out=pen_row,
                                  in_=cnt_dram.rearrange("(o s) -> o s", o=1))
                w005 = hp.tile([1, 128], f32, tag="w005", bufs=1)
                nc.vector.memset(w005, OCC_W)
                # pen broadcast: two half tiles so phase-3 adds can start early
                pen_lo = sp.tile([128, S // 2], bf16, tag="pen_lo", bufs=1)
                pen_hi = sp.tile([128, S // 2], bf16, tag="pen_hi", bufs=1)
                for n in range(NCH):
                    pen_ps = pp.tile([128, 512], f32, tag="ps")
                    nc.tensor.matmul(out=pen_ps,
                                     lhsT=w005.bitcast(f32r),
                                     rhs=pen_row[:, n * 512:(n + 1) * 512]
                                         .bitcast(f32r),
                                     start=True, stop=True,
                                     skip_group_check=True)
                    pb = pen_lo if n < NCH // 2 else pen_hi
                    off = (n % (NCH // 2)) * 512
                    nc.scalar.copy(out=pb[:, off:off + 512], in_=pen_ps)

            if stage >= 5:
                # ---- Phase 3: Dp = dA + pen, row min + exp row sums ----
                mfin = sp.tile([128, MT], f32, tag="mfin", bufs=1)
                it4 = sp.tile([128, MT], f32, tag="it4", bufs=1)
                nit4 = sp.tile([128, MT], f32, tag="nit4", bufs=1)
                bia4 = sp.tile([128, MT], f32, tag="bia4", bufs=1)
                half = sp.tile([128, MT], f32, tag="half", bufs=1)
                for m in range(MT):
                    nc.vector.tensor_tensor(out=D[m][:, 0:S // 2],
                                            in0=D[m][:, 0:S // 2], in1=pen_lo,
                                            op=Alu.add)
                    nc.vector.tensor_tensor(out=D[m][:, S // 2:S],
                                            in0=D[m][:, S // 2:S], in1=pen_hi,
                                            op=Alu.add)
                    nc.vector.tensor_reduce(
                        out=mfin[:, m:m + 1], in_=D[m],
                        axis=mybir.AxisListType.X, op=Alu.min)
                    nc.vector.tensor_scalar(out=half[:, m:m + 1],
                                            in0=mfin[:, m:m + 1], scalar1=1e-5,
                                            scalar2=H_PARAM,
                                            op0=Alu.add, op1=Alu.mult)
                    nc.vector.reciprocal(out=it4[:, m:m + 1], in_=half[:, m:m + 1])
                    nc.vector.tensor_scalar(out=nit4[:, m:m + 1],
                                            in0=it4[:, m:m + 1], scalar1=-1.0,
                                            scalar2=None, op0=Alu.mult)
                    nc.vector.tensor_tensor(out=bia4[:, m:m + 1],
                                            in0=mfin[:, m:m + 1],
                                            in1=it4[:, m:m + 1], op=Alu.mult)
                    ed = sp.tile([128, S], bf16, tag="ed")
                    nc.scalar.activation(
                        out=ed, in_=D[m],
                        func=Act.Exp,
                        bias=bia4[:, m:m + 1], scale=nit4[:, m:m + 1],
                        accum_out=sout[:, m:m + 1])
                nc.vector.tensor_copy(out=sout[:, 4:4 + MT], in_=mfin)

            nc.sync.dma_start(out=sout_ext[:], in_=sout)

    nc.compile()
    return nc


def _pack_inputs(target_features, reference_features, target_orient, refer_orient,
                 target_field, refer_field):
    iy_t, ix_t = _grid_idx(np.asarray(target_field[0], dtype=np.float32))
    iy_r, ix_r = _grid_idx(np.asarray(refer_field[0], dtype=np.float32))

    tf = _gather_patches(np.asarray(target_features[0], np.float32), iy_t, ix_t)
    rf = _gather_patches(np.asarray(reference_features[0], np.float32), iy_r, ix_r)
    to = _gather_patches(np.asarray(target_orient[0], np.float32), iy_t, ix_t)
    ro = _gather_patches(np.asarray(refer_orient[0], np.float32), iy_r, ix_r)

    FX, FY = _host_features(tf, rf, to, ro)
    FXz, FYz = _cascade_bias(FX, FY)
    FXq = (FXz * 16.0).astype(ml_dtypes.float8_e4m3).reshape(KT, 128, S)
    FYq = (FYz * 16.0).astype(ml_dtypes.float8_e4m3).reshape(KT, 128, NCH, 512)
    yw = np.ascontiguousarray(FYq.transpose(2, 1, 0, 3)).reshape(NCH, 128, KT * 512)
    in_maps = []
    for c in range(NCORES):
        xw_c = np.ascontiguousarray(
            FXq[:, :, c * MSH:(c + 1) * MSH].transpose(1, 0, 2)
        ).reshape(128, KT * 512)
        in_maps.append({"xw": xw_c, "yw": yw})
    return in_maps, (FXz, FYz)


def kernel(target_features, reference_features, target_orient, refer_orient,
           target_field, refer_field):
    global DEVICE_OK, LAST_EXEC_NS
    in_maps, (FXz, FYz) = _pack_inputs(target_features, reference_features,
                                       target_orient, refer_orient,
                                       target_field, refer_field)
    try:
        from concourse.bass_utils import run_bass_kernel_spmd
        if "nc" not in _BASS_CACHE:
            _BASS_CACHE["nc"] = _build_bass()
        nc = _BASS_CACHE["nc"]
        res = run_bass_kernel_spmd(nc, in_maps, list(range(NCORES)))
        LAST_EXEC_NS = getattr(res, "exec_time_ns", None)
        S_all = np.empty(S, np.float32)
        for c in range(NCORES):
            o = res.results[c]["sout"]  # [128, 8]
            for m in range(MT):
                S_all[c * MSH + m * 128:c * MSH + (m + 1) * 128] = o[:, m]
        DEVICE_OK = True
        loss = np.log(S_all).mean()
        return np.float32(loss)
    except Exception:
        sys.stderr.write("device path failed; host fallback\n")
        import traceback
        traceback.print_exc()
        import ml_dtypes as _mld
        FXe = (FXz * 16.0).astype(_mld.float8_e4m3).astype(np.float32) / 16.0
        FYe = (FYz * 16.0).astype(_mld.float8_e4m3).astype(np.float32) / 16.0
        dA = FXe.T @ FYe
        am = np.argmin(dA, axis=1)
        counts = np.bincount(am, minlength=S).astype(np.float32)
        dtot = dA + OCC_W * counts[None, :]
        m = dtot.min(axis=1)
        it = 1.0 / (H_PARAM * (m + 1e-5))
        Srow = np.exp((m[:, None] - dtot) * it[:, None]).sum(axis=1)
        return np.float32(np.log(Srow).mean())
